# revision 24
# baseline (speedup 1.0000x reference)
"""MoE block (top-2 routed 3x3 conv experts) Trainium2 Bass kernel — v6.

Data-parallel over batch, 2 samples per core on 8 cores. The conv is
linear in the kernel, so the top-2 expert kernels are combined with the
routing probabilities first (w_comb = sum_e p_e W_e + I, the +I folding
the residual into the center tap), then one 3x3 SAME conv per sample.
Conv-as-matmul: 6 N=391 matmuls per 6-row pair (dy-major), A-half taps
on psum partitions 0:64, center taps on 64:128 realigned +1 col by the
ACT stage, combined into [64, OBW] out batches.

v6 structure (v4 baseline 118.5us; v5 experiments: TTR crashes HW at
runtime, cross-base SBUF binary ops and 2-bank matmul outs are ISA
errors, gpsimd lacks TensorScalarPtr/TensorReduce):
- Loads x-first on all 3 DMA lanes (SP/gpsimd/ACT), weights slotted
  where they don't delay the GAP->gate->MAC critical path. Per-lane
  effective bandwidth is only ~90-130 GB/s, so s0's chunks are spread
  across all lanes and s1 streams behind them.
- PE warm-up block (~85 throwaway matmuls on wpsA) keeps the HAM clock
  gate at 8/8 through the prologue so real convs start at 2.4 GHz.
- MAC on the PE: wcomb_psum = sum_e (p_e*I)^T @ wps_e — 8 ACT ops build
  p_e*I from a shipped identity (per-partition scale), 8 accumulating
  matmuls, one ACT copy back to bf16. Replaces the serial DVE MAC chain
  (~7-9us/sample) with ~2us ACT + ~1.3us PE.
- b_comb folded into the B-half stage as the ACT activation bias; the
  combine is then a 2-operand tensor_tensor add. DVE does most pairs;
  4 late-s0 pairs go to gpsimd (tensor_tensor IS supported there) via a
  double ACT stage, relieving the DVE in the tight s0 phase.
- GAP: v4-style accumulating windows (ACT tops half / DVE the rest),
  s1's windows and gate emitted between s0 pairs as its chunks land.
"""
import os
import numpy as np
from contextlib import ExitStack

import ml_dtypes

import concourse.bass as bass
import concourse.tile as tile
from concourse import bacc, mybir
from concourse.bass_utils import run_bass_kernel_spmd
import concourse.bass_utils as _bu

if os.environ.get("KLDW") and not getattr(_bu, "_kldw_patched", False):
    _orig_run_command = _bu.run_command

    def _run_command(cmd, **kw):
        cmd = [
            c.replace("--enable-ldw-opt=false", "--enable-ldw-opt=true")
            if isinstance(c, str) else c
            for c in cmd
        ]
        return _orig_run_command(cmd, **kw)

    _bu.run_command = _run_command
    _bu._kldw_patched = True

F32 = mybir.dt.float32
BF16 = mybir.dt.bfloat16
AX = mybir.AxisListType
OP = mybir.AluOpType
ACTF = mybir.ActivationFunctionType

B, C, H, W, E, GH = 16, 64, 128, 128, 8, 16
NCORES = 8
SPB = B // NCORES          # samples per core
HP, WP = H + 2, W + 2      # 130
FLAT = HP * WP             # 16900
QC = FLAT // 4             # x-load chunk size (4225 flat elements)
OBW = 24 * WP              # out batch region width (3120)
NPAIR = 22                 # 21 six-row pairs + one trailing 2-row tile
WARM_MM = 85               # PE warm-up matmuls (span the prologue)
GPS_PAIRS_S0 = {10, 13, 16, 19}   # s0 pairs whose combine runs on gpsimd

NPBF16 = ml_dtypes.bfloat16

_cache = {}

# GAP windows over the flat layout (pad zeros included): top copy
# (partitions 0:64) covers flat[0:2QC+2), bottom copy (64:128, shifted
# +2) covers flat[2QC+2:FLAT). Each op accumulates into a part slot;
# the gate matmul's stacked wg1x2 sums the two partition halves.
GAP_TOP = [(0, QC, 0), (QC, 2 * QC + 2, 1)]
GAP_BOT = [(2 * QC, 3 * QC, 0), (3 * QC, FLAT, 1)]
QH = QC // 2
GAP_BOT4 = [
    (2 * QC, 2 * QC + QH, 0),
    (2 * QC + QH, 3 * QC, 1),
    (3 * QC, 3 * QC + QH, 2),
    (3 * QC + QH, FLAT, 3),
]


def _emit_gap_op(nc, pools, XX, part, win, is_bot, eng):
    a, b, slot = win
    lo, hi = (64, 128) if is_bot else (0, 64)
    src = XX[lo:hi, a:b]
    dst = pools["scrS" if eng == "act" else "scrD"][lo:hi]
    acc = part[lo:hi, slot : slot + 1]
    if eng == "act":
        return nc.scalar.activation(
            dst[:, 0 : b - a], src, ACTF.Copy, accum_out=acc
        )
    return nc.vector.tensor_scalar(
        dst[:, 0 : b - a], src, 0.0, 0.0, OP.add, OP.add, accum_out=acc
    )


def _emit_gate(nc, pools, s, pooled, consts, h_ext):
    """Gate MLP + softmax + top-2 for one sample (all f32).

    exp-without-max-sub (logits are small); folds the top-2 mask and
    renormalization: w8 = (u>=m2)*u / (sum((u>=m2)*u) + sum(u)*1e-8).
    Returns (wb_sb [128,E] f32 per-partition probs, b_comb [C,1]).
    """
    f = pools
    g = f["gate"]
    wg1x2_sb, bg1_sb, wg2_sb, bexp_sb, ones = consts
    n = lambda base: f"{base}{s}"

    h_ps = f["gpsum"].tile([GH, 1], F32, tag="cps", name=n("h_ps"))
    nc.tensor.matmul(h_ps[:], lhsT=wg1x2_sb, rhs=pooled[:], start=True, stop=True)
    nc.vector.tensor_scalar(h_ext[0:GH, :], h_ps[:], bg1_sb, 0.0, OP.add, OP.max)

    lg_ps = f["gpsum"].tile([1, E], F32, tag="cps", name=n("lg_ps"))
    nc.tensor.matmul(lg_ps[:], lhsT=h_ext[:], rhs=wg2_sb, start=True, stop=True)

    u = g.tile([1, E], F32, tag="u", name=n("u"))
    nc.scalar.activation(u[:], lg_ps[:], ACTF.Exp)
    usum = g.tile([1, 1], F32, tag="usum", name=n("usum"))
    nc.vector.tensor_reduce(usum[:], u[:], axis=AX.X, op=OP.add)
    m1p = g.tile([1, 1], F32, tag="m1p", name=n("m1p"))
    nc.vector.tensor_reduce(m1p[:], u[:], axis=AX.X, op=OP.max)
    pm = g.tile([1, E], F32, tag="pm", name=n("pm"))
    nc.vector.scalar_tensor_tensor(pm[:], u[:], m1p[:], u[:], op0=OP.is_lt, op1=OP.mult)
    m2 = g.tile([1, 1], F32, tag="m2", name=n("m2"))
    nc.vector.tensor_reduce(m2[:], pm[:], axis=AX.X, op=OP.max)
    spv = g.tile([1, E], F32, tag="spv", name=n("spv"))
    nc.vector.scalar_tensor_tensor(spv[:], u[:], m2[:], u[:], op0=OP.is_ge, op1=OP.mult)
    dsum = g.tile([1, 1], F32, tag="dsum", name=n("dsum"))
    nc.vector.tensor_reduce(dsum[:], spv[:], axis=AX.X, op=OP.add)
    dd = g.tile([1, 1], F32, tag="dd", name=n("dd"))
    nc.vector.scalar_tensor_tensor(dd[:], usum[:], 1e-8, dsum[:], op0=OP.mult, op1=OP.add)
    rr = g.tile([1, 1], F32, tag="rr", name=n("rr"))
    nc.vector.reciprocal(rr[:], dd[:])
    w8 = g.tile([1, E], F32, tag="w8", name=n("w8"))
    nc.vector.tensor_scalar_mul(w8[:], spv[:], rr[:])

    # broadcast w8 down all 128 partitions, then stage to SBUF for MACs
    wb_ps = f["gpsum"].tile([128, E], F32, tag="cps", name=n("wb_ps"))
    nc.tensor.matmul(wb_ps[:], lhsT=ones[:], rhs=w8[:], start=True, stop=True)
    wb_sb = g.tile([128, E], F32, tag="wb_sb", name=n("wb_sb"))
    nc.vector.tensor_copy(wb_sb[:], wb_ps[:])

    # combined bias: b_comb = b_exp^T @ w8^T
    w8c_ps = f["gpsum"].tile([E, 1], F32, tag="cps", name=n("w8c_ps"))
    nc.tensor.matmul(w8c_ps[:], lhsT=w8[:], rhs=ones[:, 0:1], start=True, stop=True)
    w8col = g.tile([E, 1], F32, tag="w8col", name=n("w8col"))
    nc.vector.tensor_copy(w8col[:], w8c_ps[:])
    bc_ps = f["gpsum"].tile([C, 1], F32, tag="cps", name=n("bc_ps"))
    nc.tensor.matmul(bc_ps[:], lhsT=bexp_sb, rhs=w8col[:], start=True, stop=True)
    b_comb = g.tile([C, 1], F32, tag="b_comb", name=n("b_comb"))
    nc.vector.tensor_copy(b_comb[:], bc_ps[:])
    return wb_sb, b_comb


def _emit_mac_pe(nc, pools, s, wb_sb, wpsA_sb, wpsB_sb, ident_sb):
    """wcomb = sum_e p_e wps_e on the PE: 8 accumulating matmuls with
    lhsT = p_e*I (built by ACT from the shipped identity with the
    per-partition probability as activation scale). Residual identity is
    pre-folded into every expert's center-tap B-half on the host."""
    f = pools
    pI = f["wcomb"].tile([128, E, 128], BF16, tag="pI", name=f"pI{s}")
    for e in range(E):
        nc.scalar.activation(
            pI[:, e, :], ident_sb[:], ACTF.Copy, scale=wb_sb[:, e : e + 1]
        )
    wcps = f["gpsum"].tile([128, 384], F32, tag="cps", name=f"wcps{s}")
    for e in range(E):
        src = wpsA_sb[:, e] if e < 4 else wpsB_sb[:, e - 4]
        nc.tensor.matmul(
            wcps[:],
            lhsT=pI[:, e, :],
            rhs=src.rearrange("p a b -> p (a b)"),
            start=(e == 0),
            stop=(e == E - 1),
        )
    wcombr = f["wcomb"].tile([128, 3, 128], BF16, tag="wcombr", name=f"wcombr{s}")
    nc.scalar.activation(
        wcombr[:].rearrange("p a b -> p (a b)"), wcps[:], ACTF.Copy
    )
    return wcombr


def _emit_pair(nc, pools, s, p, XX, wcombr, b_comb, ob, ocol, gps):
    """Conv for pair p: 6 matmuls (dy-major, N=ncol+1 so the stage's +1
    col realignment only reads written psum) into a 2-bank PSUM tile.
    ACT stages the B half with b_comb as activation bias; the combine is
    then obv = psA + stB (DVE tensor_tensor, or gpsimd via an extra ACT
    stage of the A half — gpsimd has no PSUM access)."""
    f = pools
    r0 = 6 * p
    last = p == NPAIR - 1
    nt = 1 if last else 2      # psum banks (3-row tiles) in this pair
    nr = 2 if last else 6      # rows
    ps = f["cpsum"].tile([128, 2, 512], F32, tag="cps", name=f"cps{s}_{p}")
    ncol = (nr // nt) * WP
    trows = nr // nt
    for dyi in range(3):
        for t in range(nt):
            ra = r0 + t * trows + dyi
            nc.tensor.matmul(
                ps[:, t, 0 : ncol + 1],
                lhsT=wcombr[:, dyi, :],
                rhs=XX[:, ra * WP : ra * WP + ncol + 1],
                start=(dyi == 0),
                stop=(dyi == 2),
            )
    obv = ob[:, ocol : ocol + nt * ncol].rearrange("p (t c) -> p t c", c=ncol)
    stB = f["stage"].tile([64, 2, 390], BF16, tag="stB", name=f"stB{s}_{p}")
    nc.scalar.activation(stB[:, 0:nt, 0:ncol], ps[64:128, 0:nt, 1 : ncol + 1], ACTF.Copy)
    if gps is not None:
        # gpsimd combine (TensorTensor only there, no PSUM access): ACT
        # stages the A half too; bias comes from the per-sample broadcast
        # tile in a second add
        stA = f["stage"].tile([64, 2, 390], BF16, tag="stA", name=f"stA{s}_{p}")
        nc.scalar.activation(stA[:, 0:nt, 0:ncol], ps[0:64, 0:nt, 0:ncol], ACTF.Copy)
        nc.gpsimd.tensor_tensor(
            obv, stA[:, 0:nt, 0:ncol], stB[:, 0:nt, 0:ncol], op=OP.add
        )
        return nc.gpsimd.tensor_tensor(obv, obv, gps[:, 0:nt, 0:ncol], op=OP.add)
    return nc.vector.scalar_tensor_tensor(
        obv,
        ps[0:64, 0:nt, 0:ncol],
        b_comb[:],
        stB[:, 0:nt, 0:ncol],
        op0=OP.add,
        op1=OP.add,
    )


def build_program():
    if "nc" in _cache:
        return _cache["nc"]
    nc = bacc.Bacc("TRN2", target_bir_lowering=False, debug=False, enable_asserts=False)
    xs_ap = nc.dram_tensor("xs", [SPB, 128, FLAT], BF16, kind="ExternalInput").ap()
    wpsA_d = nc.dram_tensor("wpsA", [128, E // 2, 3, 128], BF16, kind="ExternalInput").ap()
    wpsB_d = nc.dram_tensor("wpsB", [128, E // 2, 3, 128], BF16, kind="ExternalInput").ap()
    ident_d = nc.dram_tensor("ident", [128, 128], BF16, kind="ExternalInput").ap()
    gconst_d = nc.dram_tensor("gconst", [128, 90], F32, kind="ExternalInput").ap()
    out_ap = nc.dram_tensor("out", [SPB, C, H * WP], BF16, kind="ExternalOutput").ap()

    with tile.TileContext(nc) as tc, ExitStack() as ctx:
        pools = {
            "const": ctx.enter_context(tc.tile_pool(name="const", bufs=1)),
            "xx": ctx.enter_context(tc.tile_pool(name="xx", bufs=SPB)),
            "gate": ctx.enter_context(tc.tile_pool(name="gate", bufs=2)),
            "wcomb": ctx.enter_context(tc.tile_pool(name="wcomb", bufs=2)),
            "stage": ctx.enter_context(tc.tile_pool(name="stage", bufs=6)),
            "cpsum": ctx.enter_context(tc.tile_pool(name="cpsum", bufs=3, space="PSUM")),
            "gpsum": ctx.enter_context(tc.tile_pool(name="gpsum", bufs=2, space="PSUM")),
        }
        cp = pools["const"]
        # +4 zeroed pad cols so the tail tile's widened matmul read stays
        # in bounds
        XX0 = pools["xx"].tile([128, FLAT + 4], BF16, tag="XX", name="XX0")
        XX1 = pools["xx"].tile([128, FLAT + 4], BF16, tag="XX", name="XX1")
        nc.vector.memset(XX0[:, FLAT : FLAT + 4], 0.0)
        nc.vector.memset(XX1[:, FLAT : FLAT + 4], 0.0)
        gconst_sb = cp.tile([128, 90], F32)
        ones = cp.tile([1, 128], F32)
        nc.gpsimd.memset(ones[:], 1.0)
        wpsA_sb = cp.tile([128, E // 2, 3, 128], BF16)
        wpsB_sb = cp.tile([128, E // 2, 3, 128], BF16)
        ident_sb = cp.tile([128, 128], BF16)
        pools["scrD"] = cp.tile([128, QC + 2], BF16, name="scrD")
        pools["scrS"] = cp.tile([128, QC + 2], BF16, name="scrS")

        # ---- loads + prologue compute, interleaved so each consumer's
        # queue drain covers only the transfers it actually needs (a
        # consumer emitted after later triggers on a lane waits for ALL
        # of them - this drain effect, not bandwidth, dominated the v4/v6
        # prologues) ----
        C3A = 3 * QC + 2113
        nc.scalar.dma_start(wpsA_sb[:], wpsA_d[:])       # warmup needs it
        nc.scalar.dma_start(ident_sb[:], ident_d[:])

        # PE warm-up: no-DMA scratch matmuls start ~2us (HAM to 8/8),
        # then chunk-gated batches self-time the warm window to the load
        warm_sc = cp.tile([128, 384], BF16, name="warm_sc")
        nc.gpsimd.memset(warm_sc[:], 0.25)
        warm_ps = pools["gpsum"].tile([128, 384], F32, tag="cps", name="warm_ps")

        def emit_warm(n, rhs):
            for _ in range(n):
                nc.tensor.matmul(
                    warm_ps[:], lhsT=warm_sc[:, 0:128], rhs=rhs,
                    start=True, stop=True,
                )

        emit_warm(30, warm_sc[:])

        part0 = pools["gate"].tile([128, 2], F32, tag="part", name="part0")
        h_ext0 = pools["gate"].tile([GH + 1, 1], F32, tag="h_ext", name="h_ext0")
        h_ext1 = pools["gate"].tile([GH + 1, 1], F32, tag="h_ext", name="h_ext1")

        nc.sync.dma_start(XX0[:, 0:QC], xs_ap[0, :, 0:QC])
        nc.sync.dma_start(h_ext0[GH : GH + 1, 0:1], ones[0:1, 0:1])
        nc.sync.dma_start(h_ext1[GH : GH + 1, 0:1], ones[0:1, 0:1])
        _emit_gap_op(nc, pools, XX0, part0, GAP_TOP[0], is_bot=False, eng="act")
        emit_warm(8, XX0[:, 0:384])

        nc.gpsimd.dma_start(gconst_sb[:], gconst_d[:])
        nc.gpsimd.dma_start(XX0[:, QC : 2 * QC], xs_ap[0, :, QC : 2 * QC])
        nc.scalar.dma_start(XX0[:, 2 * QC : 3 * QC], xs_ap[0, :, 2 * QC : 3 * QC])
        _emit_gap_op(nc, pools, XX0, part0, GAP_TOP[1], is_bot=False, eng="dve")
        _emit_gap_op(nc, pools, XX0, part0, GAP_BOT[0], is_bot=True, eng="dve")
        emit_warm(8, XX0[:, QC : QC + 384])
        emit_warm(8, XX0[:, 2 * QC : 2 * QC + 384])

        nc.sync.dma_start(XX0[:, 3 * QC : C3A], xs_ap[0, :, 3 * QC : C3A])
        nc.gpsimd.dma_start(XX0[:, C3A:FLAT], xs_ap[0, :, C3A:FLAT])
        _emit_gap_op(nc, pools, XX0, part0, GAP_BOT[1], is_bot=True, eng="act")
        emit_warm(8, XX0[:, 3 * QC : 3 * QC + 384])

        nc.sync.dma_start(wpsB_sb[:], wpsB_d[:])

        wg1x2_sb = gconst_sb[:, 0:16]
        bg1_sb = gconst_sb[0:16, 16:17]
        wg2_sb = gconst_sb[0:17, 17:25]
        bexp_sb = gconst_sb[0:8, 25:89]
        consts = (wg1x2_sb, bg1_sb, wg2_sb, bexp_sb, ones)

        pooled0 = pools["gate"].tile([128, 1], F32, tag="pooled", name="pooled0")
        nc.vector.tensor_reduce(pooled0, part0[:], axis=AX.X, op=OP.add)
        wb0, bcomb0 = _emit_gate(nc, pools, 0, pooled0, consts, h_ext0)
        wcombr0 = _emit_mac_pe(nc, pools, 0, wb0, wpsA_sb, wpsB_sb, ident_sb)
        zb = cp.tile([64, 2, 390], BF16, name="zb")
        nc.gpsimd.memset(zb[:], 0.0)
        bB0 = pools["gate"].tile([64, 2, 390], BF16, tag="bB", name="bB0")
        nc.vector.scalar_tensor_tensor(
            bB0[:], zb[:], bcomb0[:], zb[:], op0=OP.add, op1=OP.add
        )

        # ---- s1 x loads: triggers emitted only now, after every s0
        # consumer, so no s0-side drain waits on them ----
        nc.sync.dma_start(XX1[:, 0:QC], xs_ap[1, :, 0:QC])
        nc.gpsimd.dma_start(XX1[:, QC : 2 * QC], xs_ap[1, :, QC : 2 * QC])
        nc.scalar.dma_start(XX1[:, 2 * QC : 3 * QC], xs_ap[1, :, 2 * QC : 3 * QC])
        nc.sync.dma_start(XX1[:, 3 * QC : C3A], xs_ap[1, :, 3 * QC : C3A])
        nc.gpsimd.dma_start(XX1[:, C3A:FLAT], xs_ap[1, :, C3A:FLAT])

        part1 = pools["gate"].tile([128, 4], F32, tag="part", name="part1")
        nc.gpsimd.memset(part1[0:64, 2:4], 0.0)
        s1_state = {}

        def s1_hook(p, comb):
            def pin(gi):
                tile.add_dep_helper(
                    gi.ins, comb.ins, sync=False,
                    reason="s1 prep slotted after this pair's combine",
                )
            if p == 3:
                pin(_emit_gap_op(nc, pools, XX1, part1, GAP_TOP[0], is_bot=False, eng="act"))
            elif p == 5:
                pin(_emit_gap_op(nc, pools, XX1, part1, GAP_TOP[1], is_bot=False, eng="dve"))
            elif p in (7, 9, 11, 12):
                k = {7: 0, 9: 1, 11: 2, 12: 3}[p]
                pin(_emit_gap_op(nc, pools, XX1, part1, GAP_BOT4[k], is_bot=True, eng="dve"))
            elif p == 13:
                pooled1 = pools["gate"].tile(
                    [128, 1], F32, tag="pooled", name="pooled1"
                )
                pin(nc.vector.tensor_reduce(pooled1, part1[:], axis=AX.X, op=OP.add))
                wb1, bcomb1 = _emit_gate(nc, pools, 1, pooled1, consts, h_ext1)
                s1_state["bcomb"] = bcomb1
                s1_state["wcombr"] = _emit_mac_pe(
                    nc, pools, 1, wb1, wpsA_sb, wpsB_sb, ident_sb
                )
                bB1 = pools["gate"].tile([64, 2, 390], BF16, tag="bB", name="bB1")
                nc.vector.scalar_tensor_tensor(
                    bB1[:], zb[:], bcomb1[:], zb[:], op0=OP.add, op1=OP.add
                )
                s1_state["bB"] = bB1

        # out batching: one [64, OBW] buffer per 24-row batch (batch 5 is
        # 8 rows); s0 batches drain on SP, s1 batches on gpsimd
        obstate = {0: [None, 0], 1: [None, 0]}

        bBmap = {}

        def emit_sample_pairs(s, XX, wcombr, bcomb, rng, hook=None):
            for p in rng:
                batch = min(p // 4, 5)
                ob, ocol = obstate[s]
                if ob is None:
                    ob = pools["stage"].tile(
                        [64, OBW], BF16, tag="ob", name=f"ob{s}_{batch}", bufs=3
                    )
                    obstate[s] = [ob, 0]
                    ocol = 0
                gps = bBmap.get(s) if (s == 0 and p in GPS_PAIRS_S0) else None
                comb = _emit_pair(nc, pools, s, p, XX, wcombr, bcomb, ob, ocol, gps)
                ocol += 780 if p < NPAIR - 1 else 260
                obstate[s][1] = ocol
                bcols = OBW if batch < 5 else 1040
                if ocol == bcols:
                    lane = nc.sync if s == 0 else nc.gpsimd
                    lane.dma_start(
                        out_ap[s, :, 24 * batch * WP : 24 * batch * WP + bcols],
                        ob[:, 0:bcols],
                    )
                    obstate[s] = [None, 0]
                if hook is not None:
                    hook(p, comb)

        bBmap[0] = bB0
        emit_sample_pairs(0, XX0, wcombr0, bcomb0, range(NPAIR), s1_hook)
        emit_sample_pairs(
            1, XX1, s1_state["wcombr"], s1_state["bcomb"], range(NPAIR)
        )

    nc.compile()
    _cache["nc"] = nc
    return nc


def host_prep(x, wg1, bg1, wg2, bg2, w_exp, b_exp):
    """Host-side layout prep + per-core sharding. Returns in_maps list."""
    x = np.asarray(x, dtype=np.float32)
    wg1 = np.asarray(wg1, dtype=np.float32)
    bg1 = np.asarray(bg1, dtype=np.float32)
    wg2 = np.asarray(wg2, dtype=np.float32)
    bg2 = np.asarray(bg2, dtype=np.float32)
    w_exp = np.asarray(w_exp, dtype=np.float32)
    b_exp = np.asarray(b_exp, dtype=np.float32)

    # x shipped as [B, 128, FLAT] bf16: rows 0:64 = zero-padded flat
    # image, rows 64:128 = the same shifted +2 elements (the conv's
    # bottom-half K copy) — both SBUF halves land in one full-rate DMA
    xpad = np.zeros((B, C, HP, WP), np.float32)
    xpad[:, :, 1 : H + 1, 1 : W + 1] = x
    flat = xpad.reshape(B, C, FLAT)
    xs = np.zeros((B, 128, FLAT), NPBF16)
    xs[:, 0:64] = flat.astype(NPBF16)
    xs[:, 64:128, 0 : FLAT - 2] = flat[:, :, 2:].astype(NPBF16)

    # wps [128, E, 3(dy), 128]: K top/bottom = taps dx 0/2 on M 0:64 (A),
    # center dx=1 on M 64:128 top (B, bottom zero). Residual identity is
    # folded into every expert's center tap (sum of probs is ~1).
    wt = np.transpose(w_exp, (2, 0, 3, 4, 1))  # [I, E, dy, dx, O]
    wps = np.zeros((128, E, 3, 128), np.float32)
    wps[0:64, :, :, 0:64] = wt[:, :, :, 0, :]
    wps[64:128, :, :, 0:64] = wt[:, :, :, 2, :]
    wps[0:64, :, :, 64:128] = wt[:, :, :, 1, :]
    ii = np.arange(64)
    wps[ii, :, 1, 64 + ii] += 1.0

    gconst = np.zeros((128, 90), np.float32)
    gconst[:, 0:16] = np.concatenate([wg1, wg1], axis=0) / (H * W)
    gconst[0:16, 16] = bg1
    gconst[0:16, 17:25] = wg2
    gconst[16, 17:25] = bg2
    gconst[0:8, 25:89] = b_exp

    shared = {
        "wpsA": np.ascontiguousarray(wps[:, 0:4]).astype(NPBF16),
        "wpsB": np.ascontiguousarray(wps[:, 4:8]).astype(NPBF16),
        "ident": np.eye(128, dtype=NPBF16),
        "gconst": gconst,
    }
    return [
        {"xs": np.ascontiguousarray(xs[SPB * k : SPB * (k + 1)]), **shared}
        for k in range(NCORES)
    ]


def _decode_out(o):
    """[C, H*WP] bf16 -> [C, H, W] f32 (strip the pad columns)."""
    return np.asarray(o, dtype=np.float32).reshape(C, H, WP)[:, :, 0:W]


def kernel(x, wg1, bg1, wg2, bg2, w_exp, b_exp):
    nc = build_program()
    in_maps = host_prep(x, wg1, bg1, wg2, bg2, w_exp, b_exp)
    res = run_bass_kernel_spmd(nc, in_maps, list(range(NCORES)))
    out = np.empty((B, C, H, W), np.float32)
    for k in range(NCORES):
        o = np.asarray(res.results[k]["out"])
        for s in range(SPB):
            out[SPB * k + s] = _decode_out(o[s])
    return out


# revision 25
# speedup vs baseline: 1.0416x; 1.0416x over previous
"""MoE block (top-2 routed 3x3 conv experts) Trainium2 Bass kernel — v6.

Data-parallel over batch, 2 samples per core on 8 cores. The conv is
linear in the kernel, so the top-2 expert kernels are combined with the
routing probabilities first (w_comb = sum_e p_e W_e + I, the +I folding
the residual into the center tap), then one 3x3 SAME conv per sample.
Conv-as-matmul: 6 N=391 matmuls per 6-row pair (dy-major), A-half taps
on psum partitions 0:64, center taps on 64:128 realigned +1 col by the
ACT stage, combined into [64, OBW] out batches.

v6 structure (v4 baseline 118.5us; v5 experiments: TTR crashes HW at
runtime, cross-base SBUF binary ops and 2-bank matmul outs are ISA
errors, gpsimd lacks TensorScalarPtr/TensorReduce):
- Loads x-first on all 3 DMA lanes (SP/gpsimd/ACT), weights slotted
  where they don't delay the GAP->gate->MAC critical path. Per-lane
  effective bandwidth is only ~90-130 GB/s, so s0's chunks are spread
  across all lanes and s1 streams behind them.
- PE warm-up block (~85 throwaway matmuls on wpsA) keeps the HAM clock
  gate at 8/8 through the prologue so real convs start at 2.4 GHz.
- MAC on the PE: wcomb_psum = sum_e (p_e*I)^T @ wps_e — 8 ACT ops build
  p_e*I from a shipped identity (per-partition scale), 8 accumulating
  matmuls, one ACT copy back to bf16. Replaces the serial DVE MAC chain
  (~7-9us/sample) with ~2us ACT + ~1.3us PE.
- b_comb folded into the B-half stage as the ACT activation bias; the
  combine is then a 2-operand tensor_tensor add. DVE does most pairs;
  4 late-s0 pairs go to gpsimd (tensor_tensor IS supported there) via a
  double ACT stage, relieving the DVE in the tight s0 phase.
- GAP: v4-style accumulating windows (ACT tops half / DVE the rest),
  s1's windows and gate emitted between s0 pairs as its chunks land.
"""
import os
import numpy as np
from contextlib import ExitStack

import ml_dtypes

import concourse.bass as bass
import concourse.tile as tile
from concourse import bacc, mybir
from concourse.bass_utils import run_bass_kernel_spmd
import concourse.bass_utils as _bu

if os.environ.get("KLDW") and not getattr(_bu, "_kldw_patched", False):
    _orig_run_command = _bu.run_command

    def _run_command(cmd, **kw):
        cmd = [
            c.replace("--enable-ldw-opt=false", "--enable-ldw-opt=true")
            if isinstance(c, str) else c
            for c in cmd
        ]
        return _orig_run_command(cmd, **kw)

    _bu.run_command = _run_command
    _bu._kldw_patched = True

F32 = mybir.dt.float32
BF16 = mybir.dt.bfloat16
AX = mybir.AxisListType
OP = mybir.AluOpType
ACTF = mybir.ActivationFunctionType

B, C, H, W, E, GH = 16, 64, 128, 128, 8, 16
NCORES = 8
SPB = B // NCORES          # samples per core
HP, WP = H + 2, W + 2      # 130
FLAT = HP * WP             # 16900
QC = FLAT // 4             # x-load chunk size (4225 flat elements)
OBW = 24 * WP              # out batch region width (3120)
NPAIR = 22                 # 21 six-row pairs + one trailing 2-row tile
WARM_MM = 85               # PE warm-up matmuls (span the prologue)
GPS_PAIRS_S0 = {10, 13, 16, 19}   # s0 pairs whose combine runs on gpsimd

NPBF16 = ml_dtypes.bfloat16

_cache = {}

# GAP windows over the flat layout (pad zeros included): top copy
# (partitions 0:64) covers flat[0:2QC+2), bottom copy (64:128, shifted
# +2) covers flat[2QC+2:FLAT). Each op accumulates into a part slot;
# the gate matmul's stacked wg1x2 sums the two partition halves.
GAP_TOP = [(0, QC, 0), (QC, 2 * QC + 2, 1)]
GAP_BOT = [(2 * QC, 3 * QC, 0), (3 * QC, FLAT, 1)]
QH = QC // 2
GAP_BOT4 = [
    (2 * QC, 2 * QC + QH, 0),
    (2 * QC + QH, 3 * QC, 1),
    (3 * QC, 3 * QC + QH, 2),
    (3 * QC + QH, FLAT, 3),
]


def _emit_gap_op(nc, pools, XX, part, win, is_bot, eng):
    a, b, slot = win
    lo, hi = (64, 128) if is_bot else (0, 64)
    src = XX[lo:hi, a:b]
    dst = pools["scrS" if eng == "act" else "scrD"][lo:hi]
    acc = part[lo:hi, slot : slot + 1]
    if eng == "act":
        return nc.scalar.activation(
            dst[:, 0 : b - a], src, ACTF.Copy, accum_out=acc
        )
    return nc.vector.tensor_scalar(
        dst[:, 0 : b - a], src, 0.0, 0.0, OP.add, OP.add, accum_out=acc
    )


def _emit_gate(nc, pools, s, pooled, consts, h_ext):
    """Gate MLP + softmax + top-2 for one sample (all f32).

    exp-without-max-sub (logits are small); folds the top-2 mask and
    renormalization: w8 = (u>=m2)*u / (sum((u>=m2)*u) + sum(u)*1e-8).
    Returns (wb_sb [128,E] f32 per-partition probs, b_comb [C,1]).
    """
    f = pools
    g = f["gate"]
    wg1x2_sb, bg1_sb, wg2_sb, bexp_sb, ones = consts
    n = lambda base: f"{base}{s}"

    h_ps = f["gpsum"].tile([GH, 1], F32, tag="cps", name=n("h_ps"))
    nc.tensor.matmul(h_ps[:], lhsT=wg1x2_sb, rhs=pooled[:], start=True, stop=True)
    nc.vector.tensor_scalar(h_ext[0:GH, :], h_ps[:], bg1_sb, 0.0, OP.add, OP.max)

    lg_ps = f["gpsum"].tile([1, E], F32, tag="cps", name=n("lg_ps"))
    nc.tensor.matmul(lg_ps[:], lhsT=h_ext[:], rhs=wg2_sb, start=True, stop=True)

    u = g.tile([1, E], F32, tag="u", name=n("u"))
    nc.scalar.activation(u[:], lg_ps[:], ACTF.Exp)
    usum = g.tile([1, 1], F32, tag="usum", name=n("usum"))
    nc.vector.tensor_reduce(usum[:], u[:], axis=AX.X, op=OP.add)
    m1p = g.tile([1, 1], F32, tag="m1p", name=n("m1p"))
    nc.vector.tensor_reduce(m1p[:], u[:], axis=AX.X, op=OP.max)
    pm = g.tile([1, E], F32, tag="pm", name=n("pm"))
    nc.vector.scalar_tensor_tensor(pm[:], u[:], m1p[:], u[:], op0=OP.is_lt, op1=OP.mult)
    m2 = g.tile([1, 1], F32, tag="m2", name=n("m2"))
    nc.vector.tensor_reduce(m2[:], pm[:], axis=AX.X, op=OP.max)
    spv = g.tile([1, E], F32, tag="spv", name=n("spv"))
    nc.vector.scalar_tensor_tensor(spv[:], u[:], m2[:], u[:], op0=OP.is_ge, op1=OP.mult)
    dsum = g.tile([1, 1], F32, tag="dsum", name=n("dsum"))
    nc.vector.tensor_reduce(dsum[:], spv[:], axis=AX.X, op=OP.add)
    dd = g.tile([1, 1], F32, tag="dd", name=n("dd"))
    nc.vector.scalar_tensor_tensor(dd[:], usum[:], 1e-8, dsum[:], op0=OP.mult, op1=OP.add)
    rr = g.tile([1, 1], F32, tag="rr", name=n("rr"))
    nc.vector.reciprocal(rr[:], dd[:])
    w8 = g.tile([1, E], F32, tag="w8", name=n("w8"))
    nc.vector.tensor_scalar_mul(w8[:], spv[:], rr[:])

    # broadcast w8 down all 128 partitions, then stage to SBUF for MACs
    wb_ps = f["gpsum"].tile([128, E], F32, tag="cps", name=n("wb_ps"))
    nc.tensor.matmul(wb_ps[:], lhsT=ones[:], rhs=w8[:], start=True, stop=True)
    wb_sb = g.tile([128, E], F32, tag="wb_sb", name=n("wb_sb"))
    nc.vector.tensor_copy(wb_sb[:], wb_ps[:])

    # combined bias: b_comb = b_exp^T @ w8^T
    w8c_ps = f["gpsum"].tile([E, 1], F32, tag="cps", name=n("w8c_ps"))
    nc.tensor.matmul(w8c_ps[:], lhsT=w8[:], rhs=ones[:, 0:1], start=True, stop=True)
    w8col = g.tile([E, 1], F32, tag="w8col", name=n("w8col"))
    nc.vector.tensor_copy(w8col[:], w8c_ps[:])
    bc_ps = f["gpsum"].tile([C, 1], F32, tag="cps", name=n("bc_ps"))
    nc.tensor.matmul(bc_ps[:], lhsT=bexp_sb, rhs=w8col[:], start=True, stop=True)
    b_comb = g.tile([C, 1], F32, tag="b_comb", name=n("b_comb"))
    nc.vector.tensor_copy(b_comb[:], bc_ps[:])
    return wb_sb, b_comb


def _emit_mac_pe(nc, pools, s, wb_sb, wpsA_sb, wpsB_sb, ident_sb):
    """wcomb = sum_e p_e wps_e on the PE: 8 accumulating matmuls with
    lhsT = p_e*I (built by ACT from the shipped identity with the
    per-partition probability as activation scale). Residual identity is
    pre-folded into every expert's center-tap B-half on the host."""
    f = pools
    pI = f["wcomb"].tile([128, E, 128], BF16, tag="pI", name=f"pI{s}")
    for e in range(E):
        nc.scalar.activation(
            pI[:, e, :], ident_sb[:], ACTF.Copy, scale=wb_sb[:, e : e + 1]
        )
    wcps = f["gpsum"].tile([128, 384], F32, tag="cps", name=f"wcps{s}")
    for e in range(E):
        src = wpsA_sb[:, e] if e < 4 else wpsB_sb[:, e - 4]
        nc.tensor.matmul(
            wcps[:],
            lhsT=pI[:, e, :],
            rhs=src.rearrange("p a b -> p (a b)"),
            start=(e == 0),
            stop=(e == E - 1),
        )
    wcombr = f["wcomb"].tile([128, 3, 128], BF16, tag="wcombr", name=f"wcombr{s}")
    nc.scalar.activation(
        wcombr[:].rearrange("p a b -> p (a b)"), wcps[:], ACTF.Copy
    )
    return wcombr


def _emit_pair(nc, pools, s, p, XX, wcombr, b_comb, ob, ocol, gps):
    """Conv for pair p: 6 matmuls (dy-major, N=ncol+1 so the stage's +1
    col realignment only reads written psum) into a 2-bank PSUM tile.
    ACT stages the B half with b_comb as activation bias; the combine is
    then obv = psA + stB (DVE tensor_tensor, or gpsimd via an extra ACT
    stage of the A half — gpsimd has no PSUM access)."""
    f = pools
    r0 = 6 * p
    last = p == NPAIR - 1
    nt = 1 if last else 2      # psum banks (3-row tiles) in this pair
    nr = 2 if last else 6      # rows
    ps = f["cpsum"].tile([128, 2, 512], F32, tag="cps", name=f"cps{s}_{p}")
    ncol = (nr // nt) * WP
    trows = nr // nt
    for dyi in range(3):
        for t in range(nt):
            ra = r0 + t * trows + dyi
            nc.tensor.matmul(
                ps[:, t, 0 : ncol + 1],
                lhsT=wcombr[:, dyi, :],
                rhs=XX[:, ra * WP : ra * WP + ncol + 1],
                start=(dyi == 0),
                stop=(dyi == 2),
            )
    obv = ob[:, ocol : ocol + nt * ncol].rearrange("p (t c) -> p t c", c=ncol)
    stB = f["stage"].tile([64, 2, 390], BF16, tag="stB", name=f"stB{s}_{p}")
    nc.scalar.activation(stB[:, 0:nt, 0:ncol], ps[64:128, 0:nt, 1 : ncol + 1], ACTF.Copy)
    if gps is not None:
        # gpsimd combine (TensorTensor only there, no PSUM access): ACT
        # stages the A half too; bias comes from the per-sample broadcast
        # tile in a second add
        stA = f["stage"].tile([64, 2, 390], BF16, tag="stA", name=f"stA{s}_{p}")
        nc.scalar.activation(stA[:, 0:nt, 0:ncol], ps[0:64, 0:nt, 0:ncol], ACTF.Copy)
        nc.gpsimd.tensor_tensor(
            obv, stA[:, 0:nt, 0:ncol], stB[:, 0:nt, 0:ncol], op=OP.add
        )
        return nc.gpsimd.tensor_tensor(obv, obv, gps[:, 0:nt, 0:ncol], op=OP.add)
    return nc.vector.scalar_tensor_tensor(
        obv,
        ps[0:64, 0:nt, 0:ncol],
        b_comb[:],
        stB[:, 0:nt, 0:ncol],
        op0=OP.add,
        op1=OP.add,
    )


def build_program():
    if "nc" in _cache:
        return _cache["nc"]
    nc = bacc.Bacc("TRN2", target_bir_lowering=False, debug=False, enable_asserts=False)
    xs_ap = nc.dram_tensor("xs", [SPB, 128, FLAT], BF16, kind="ExternalInput").ap()
    wpsA_d = nc.dram_tensor("wpsA", [128, E // 2, 3, 128], BF16, kind="ExternalInput").ap()
    wpsB_d = nc.dram_tensor("wpsB", [128, E // 2, 3, 128], BF16, kind="ExternalInput").ap()
    ident_d = nc.dram_tensor("ident", [128, 128], BF16, kind="ExternalInput").ap()
    gconst_d = nc.dram_tensor("gconst", [128, 90], F32, kind="ExternalInput").ap()
    out_ap = nc.dram_tensor("out", [SPB, C, H * WP], BF16, kind="ExternalOutput").ap()

    with tile.TileContext(nc) as tc, ExitStack() as ctx:
        pools = {
            "const": ctx.enter_context(tc.tile_pool(name="const", bufs=1)),
            "xx": ctx.enter_context(tc.tile_pool(name="xx", bufs=SPB)),
            "gate": ctx.enter_context(tc.tile_pool(name="gate", bufs=2)),
            "wcomb": ctx.enter_context(tc.tile_pool(name="wcomb", bufs=2)),
            "stage": ctx.enter_context(tc.tile_pool(name="stage", bufs=6)),
            "cpsum": ctx.enter_context(tc.tile_pool(name="cpsum", bufs=3, space="PSUM")),
            "gpsum": ctx.enter_context(tc.tile_pool(name="gpsum", bufs=2, space="PSUM")),
        }
        cp = pools["const"]
        # +4 zeroed pad cols so the tail tile's widened matmul read stays
        # in bounds
        XX0 = pools["xx"].tile([128, FLAT + 4], BF16, tag="XX", name="XX0")
        XX1 = pools["xx"].tile([128, FLAT + 4], BF16, tag="XX", name="XX1")
        nc.vector.memset(XX0[:, FLAT : FLAT + 4], 0.0)
        nc.vector.memset(XX1[:, FLAT : FLAT + 4], 0.0)
        gconst_sb = cp.tile([128, 90], F32)
        ones = cp.tile([1, 128], F32)
        nc.gpsimd.memset(ones[:], 1.0)
        wpsA_sb = cp.tile([128, E // 2, 3, 128], BF16)
        wpsB_sb = cp.tile([128, E // 2, 3, 128], BF16)
        ident_sb = cp.tile([128, 128], BF16)
        pools["scrD"] = cp.tile([128, QC + 2], BF16, name="scrD")
        pools["scrS"] = cp.tile([128, QC + 2], BF16, name="scrS")

        # ---- loads + prologue compute, interleaved so each consumer's
        # queue drain covers only the transfers it actually needs (a
        # consumer emitted after later triggers on a lane waits for ALL
        # of them - this drain effect, not bandwidth, dominated the v4/v6
        # prologues) ----
        C3A = 3 * QC + 2113
        nc.scalar.dma_start(wpsA_sb[:], wpsA_d[:])       # warmup needs it
        nc.scalar.dma_start(ident_sb[:], ident_d[:])

        # PE warm-up: no-DMA scratch matmuls start ~2us (HAM to 8/8),
        # then chunk-gated batches self-time the warm window to the load
        warm_sc = cp.tile([128, 384], BF16, name="warm_sc")
        nc.gpsimd.memset(warm_sc[:], 0.25)
        warm_ps = pools["gpsum"].tile([128, 384], F32, tag="cps", name="warm_ps")

        def emit_warm(n, rhs):
            for _ in range(n):
                nc.tensor.matmul(
                    warm_ps[:], lhsT=warm_sc[:, 0:128], rhs=rhs,
                    start=True, stop=True,
                )

        emit_warm(30, warm_sc[:])

        part0 = pools["gate"].tile([128, 2], F32, tag="part", name="part0")
        h_ext0 = pools["gate"].tile([GH + 1, 1], F32, tag="h_ext", name="h_ext0")
        h_ext1 = pools["gate"].tile([GH + 1, 1], F32, tag="h_ext", name="h_ext1")

        nc.sync.dma_start(XX0[:, 0:QC], xs_ap[0, :, 0:QC])
        nc.sync.dma_start(h_ext0[GH : GH + 1, 0:1], ones[0:1, 0:1])
        nc.sync.dma_start(h_ext1[GH : GH + 1, 0:1], ones[0:1, 0:1])
        _emit_gap_op(nc, pools, XX0, part0, GAP_TOP[0], is_bot=False, eng="act")
        emit_warm(8, XX0[:, 0:384])

        nc.gpsimd.dma_start(gconst_sb[:], gconst_d[:])
        nc.sync.dma_start(XX0[:, QC : 2 * QC], xs_ap[0, :, QC : 2 * QC])
        nc.scalar.dma_start(XX0[:, 2 * QC : 3 * QC], xs_ap[0, :, 2 * QC : 3 * QC])
        _emit_gap_op(nc, pools, XX0, part0, GAP_TOP[1], is_bot=False, eng="dve")
        _emit_gap_op(nc, pools, XX0, part0, GAP_BOT[0], is_bot=True, eng="dve")
        emit_warm(8, XX0[:, QC : QC + 384])
        emit_warm(8, XX0[:, 2 * QC : 2 * QC + 384])

        nc.gpsimd.dma_start(XX0[:, C3A:FLAT], xs_ap[0, :, C3A:FLAT])
        nc.sync.dma_start(XX0[:, 3 * QC : C3A], xs_ap[0, :, 3 * QC : C3A])
        _emit_gap_op(nc, pools, XX0, part0, GAP_BOT[1], is_bot=True, eng="act")
        emit_warm(8, XX0[:, 3 * QC : 3 * QC + 384])

        nc.scalar.dma_start(wpsB_sb[:], wpsB_d[:])

        wg1x2_sb = gconst_sb[:, 0:16]
        bg1_sb = gconst_sb[0:16, 16:17]
        wg2_sb = gconst_sb[0:17, 17:25]
        bexp_sb = gconst_sb[0:8, 25:89]
        consts = (wg1x2_sb, bg1_sb, wg2_sb, bexp_sb, ones)

        pooled0 = pools["gate"].tile([128, 1], F32, tag="pooled", name="pooled0")
        nc.vector.tensor_reduce(pooled0, part0[:], axis=AX.X, op=OP.add)
        wb0, bcomb0 = _emit_gate(nc, pools, 0, pooled0, consts, h_ext0)
        wcombr0 = _emit_mac_pe(nc, pools, 0, wb0, wpsA_sb, wpsB_sb, ident_sb)
        zb = cp.tile([64, 2, 390], BF16, name="zb")
        nc.gpsimd.memset(zb[:], 0.0)
        bB0 = pools["gate"].tile([64, 2, 390], BF16, tag="bB", name="bB0")
        bB0i = nc.vector.scalar_tensor_tensor(
            bB0[:], zb[:], bcomb0[:], zb[:], op0=OP.add, op1=OP.add
        )

        # ---- s1 x loads: all on the gpsimd lane (free after gconst/c3b)
        nc.gpsimd.dma_start(XX1[:, 0:QC], xs_ap[1, :, 0:QC])
        nc.gpsimd.dma_start(XX1[:, QC : 2 * QC], xs_ap[1, :, QC : 2 * QC])
        nc.gpsimd.dma_start(XX1[:, 2 * QC : 3 * QC], xs_ap[1, :, 2 * QC : 3 * QC])
        nc.gpsimd.dma_start(XX1[:, 3 * QC : FLAT], xs_ap[1, :, 3 * QC : FLAT])

        part1 = pools["gate"].tile([128, 4], F32, tag="part", name="part1")
        nc.gpsimd.memset(part1[0:64, 2:4], 0.0)
        s1_state = {}

        def s1_hook(p, comb):
            def pin(gi):
                tile.add_dep_helper(
                    gi.ins, bB0i.ins, sync=False,
                    reason="s1 prep ordered after the s0 gate block",
                )
            if p == 3:
                pin(_emit_gap_op(nc, pools, XX1, part1, GAP_TOP[0], is_bot=False, eng="act"))
            elif p == 5:
                pin(_emit_gap_op(nc, pools, XX1, part1, GAP_TOP[1], is_bot=False, eng="dve"))
            elif p in (7, 9, 11, 12):
                k = {7: 0, 9: 1, 11: 2, 12: 3}[p]
                pin(_emit_gap_op(nc, pools, XX1, part1, GAP_BOT4[k], is_bot=True, eng="dve"))
            elif p == 13:
                pooled1 = pools["gate"].tile(
                    [128, 1], F32, tag="pooled", name="pooled1"
                )
                pin(nc.vector.tensor_reduce(pooled1, part1[:], axis=AX.X, op=OP.add))
                wb1, bcomb1 = _emit_gate(nc, pools, 1, pooled1, consts, h_ext1)
                s1_state["bcomb"] = bcomb1
                s1_state["wcombr"] = _emit_mac_pe(
                    nc, pools, 1, wb1, wpsA_sb, wpsB_sb, ident_sb
                )
                bB1 = pools["gate"].tile([64, 2, 390], BF16, tag="bB", name="bB1")
                nc.vector.scalar_tensor_tensor(
                    bB1[:], zb[:], bcomb1[:], zb[:], op0=OP.add, op1=OP.add
                )
                s1_state["bB"] = bB1

        # out batching: one [64, OBW] buffer per 24-row batch (batch 5 is
        # 8 rows); s0 batches drain on SP, s1 batches on gpsimd
        obstate = {0: [None, 0], 1: [None, 0]}

        bBmap = {}

        def emit_sample_pairs(s, XX, wcombr, bcomb, rng, hook=None):
            for p in rng:
                batch = min(p // 4, 5)
                ob, ocol = obstate[s]
                if ob is None:
                    ob = pools["stage"].tile(
                        [64, OBW], BF16, tag="ob", name=f"ob{s}_{batch}", bufs=3
                    )
                    obstate[s] = [ob, 0]
                    ocol = 0
                gps = bBmap.get(s) if (s == 0 and p in GPS_PAIRS_S0) else None
                comb = _emit_pair(nc, pools, s, p, XX, wcombr, bcomb, ob, ocol, gps)
                ocol += 780 if p < NPAIR - 1 else 260
                obstate[s][1] = ocol
                bcols = OBW if batch < 5 else 1040
                if ocol == bcols:
                    lane = nc.sync if s == 0 else nc.gpsimd
                    lane.dma_start(
                        out_ap[s, :, 24 * batch * WP : 24 * batch * WP + bcols],
                        ob[:, 0:bcols],
                    )
                    obstate[s] = [None, 0]
                if hook is not None:
                    hook(p, comb)

        bBmap[0] = bB0
        emit_sample_pairs(0, XX0, wcombr0, bcomb0, range(NPAIR), s1_hook)
        emit_sample_pairs(
            1, XX1, s1_state["wcombr"], s1_state["bcomb"], range(NPAIR)
        )

    nc.compile()
    _cache["nc"] = nc
    return nc


def host_prep(x, wg1, bg1, wg2, bg2, w_exp, b_exp):
    """Host-side layout prep + per-core sharding. Returns in_maps list."""
    x = np.asarray(x, dtype=np.float32)
    wg1 = np.asarray(wg1, dtype=np.float32)
    bg1 = np.asarray(bg1, dtype=np.float32)
    wg2 = np.asarray(wg2, dtype=np.float32)
    bg2 = np.asarray(bg2, dtype=np.float32)
    w_exp = np.asarray(w_exp, dtype=np.float32)
    b_exp = np.asarray(b_exp, dtype=np.float32)

    # x shipped as [B, 128, FLAT] bf16: rows 0:64 = zero-padded flat
    # image, rows 64:128 = the same shifted +2 elements (the conv's
    # bottom-half K copy) — both SBUF halves land in one full-rate DMA
    xpad = np.zeros((B, C, HP, WP), np.float32)
    xpad[:, :, 1 : H + 1, 1 : W + 1] = x
    flat = xpad.reshape(B, C, FLAT)
    xs = np.zeros((B, 128, FLAT), NPBF16)
    xs[:, 0:64] = flat.astype(NPBF16)
    xs[:, 64:128, 0 : FLAT - 2] = flat[:, :, 2:].astype(NPBF16)

    # wps [128, E, 3(dy), 128]: K top/bottom = taps dx 0/2 on M 0:64 (A),
    # center dx=1 on M 64:128 top (B, bottom zero). Residual identity is
    # folded into every expert's center tap (sum of probs is ~1).
    wt = np.transpose(w_exp, (2, 0, 3, 4, 1))  # [I, E, dy, dx, O]
    wps = np.zeros((128, E, 3, 128), np.float32)
    wps[0:64, :, :, 0:64] = wt[:, :, :, 0, :]
    wps[64:128, :, :, 0:64] = wt[:, :, :, 2, :]
    wps[0:64, :, :, 64:128] = wt[:, :, :, 1, :]
    ii = np.arange(64)
    wps[ii, :, 1, 64 + ii] += 1.0

    gconst = np.zeros((128, 90), np.float32)
    gconst[:, 0:16] = np.concatenate([wg1, wg1], axis=0) / (H * W)
    gconst[0:16, 16] = bg1
    gconst[0:16, 17:25] = wg2
    gconst[16, 17:25] = bg2
    gconst[0:8, 25:89] = b_exp

    shared = {
        "wpsA": np.ascontiguousarray(wps[:, 0:4]).astype(NPBF16),
        "wpsB": np.ascontiguousarray(wps[:, 4:8]).astype(NPBF16),
        "ident": np.eye(128, dtype=NPBF16),
        "gconst": gconst,
    }
    return [
        {"xs": np.ascontiguousarray(xs[SPB * k : SPB * (k + 1)]), **shared}
        for k in range(NCORES)
    ]


def _decode_out(o):
    """[C, H*WP] bf16 -> [C, H, W] f32 (strip the pad columns)."""
    return np.asarray(o, dtype=np.float32).reshape(C, H, WP)[:, :, 0:W]


def kernel(x, wg1, bg1, wg2, bg2, w_exp, b_exp):
    nc = build_program()
    in_maps = host_prep(x, wg1, bg1, wg2, bg2, w_exp, b_exp)
    res = run_bass_kernel_spmd(nc, in_maps, list(range(NCORES)))
    out = np.empty((B, C, H, W), np.float32)
    for k in range(NCORES):
        o = np.asarray(res.results[k]["out"])
        for s in range(SPB):
            out[SPB * k + s] = _decode_out(o[s])
    return out


# revision 29
# speedup vs baseline: 1.0713x; 1.0286x over previous
"""MoE block (top-2 routed 3x3 conv experts) Trainium2 Bass kernel — v6.

Data-parallel over batch, 2 samples per core on 8 cores. The conv is
linear in the kernel, so the top-2 expert kernels are combined with the
routing probabilities first (w_comb = sum_e p_e W_e + I, the +I folding
the residual into the center tap), then one 3x3 SAME conv per sample.
Conv-as-matmul: 6 N=391 matmuls per 6-row pair (dy-major), A-half taps
on psum partitions 0:64, center taps on 64:128 realigned +1 col by the
ACT stage, combined into [64, OBW] out batches.

v6 structure (v4 baseline 118.5us; v5 experiments: TTR crashes HW at
runtime, cross-base SBUF binary ops and 2-bank matmul outs are ISA
errors, gpsimd lacks TensorScalarPtr/TensorReduce):
- Loads x-first on all 3 DMA lanes (SP/gpsimd/ACT), weights slotted
  where they don't delay the GAP->gate->MAC critical path. Per-lane
  effective bandwidth is only ~90-130 GB/s, so s0's chunks are spread
  across all lanes and s1 streams behind them.
- PE warm-up block (~85 throwaway matmuls on wpsA) keeps the HAM clock
  gate at 8/8 through the prologue so real convs start at 2.4 GHz.
- MAC on the PE: wcomb_psum = sum_e (p_e*I)^T @ wps_e — 8 ACT ops build
  p_e*I from a shipped identity (per-partition scale), 8 accumulating
  matmuls, one ACT copy back to bf16. Replaces the serial DVE MAC chain
  (~7-9us/sample) with ~2us ACT + ~1.3us PE.
- b_comb folded into the B-half stage as the ACT activation bias; the
  combine is then a 2-operand tensor_tensor add. DVE does most pairs;
  4 late-s0 pairs go to gpsimd (tensor_tensor IS supported there) via a
  double ACT stage, relieving the DVE in the tight s0 phase.
- GAP: v4-style accumulating windows (ACT tops half / DVE the rest),
  s1's windows and gate emitted between s0 pairs as its chunks land.
"""
import os
import numpy as np
from contextlib import ExitStack

import ml_dtypes

import concourse.bass as bass
import concourse.tile as tile
from concourse import bacc, mybir
from concourse.bass_utils import run_bass_kernel_spmd
import concourse.bass_utils as _bu

if os.environ.get("KLDW") and not getattr(_bu, "_kldw_patched", False):
    _orig_run_command = _bu.run_command

    def _run_command(cmd, **kw):
        cmd = [
            c.replace("--enable-ldw-opt=false", "--enable-ldw-opt=true")
            if isinstance(c, str) else c
            for c in cmd
        ]
        return _orig_run_command(cmd, **kw)

    _bu.run_command = _run_command
    _bu._kldw_patched = True

F32 = mybir.dt.float32
BF16 = mybir.dt.bfloat16
AX = mybir.AxisListType
OP = mybir.AluOpType
ACTF = mybir.ActivationFunctionType

B, C, H, W, E, GH = 16, 64, 128, 128, 8, 16
NCORES = 8
SPB = B // NCORES          # samples per core
HP, WP = H + 2, W + 2      # 130
FLAT = HP * WP             # 16900
QC = FLAT // 4             # x-load chunk size (4225 flat elements)
OBW = 24 * WP              # out batch region width (3120)
NPAIR = 22                 # 21 six-row pairs + one trailing 2-row tile
WARM_MM = 85               # PE warm-up matmuls (span the prologue)
GPS_PAIRS_S0 = {10, 13, 16, 19}   # s0 pairs whose combine runs on gpsimd

NPBF16 = ml_dtypes.bfloat16

_cache = {}

# GAP windows over the flat layout (pad zeros included): top copy
# (partitions 0:64) covers flat[0:2QC+2), bottom copy (64:128, shifted
# +2) covers flat[2QC+2:FLAT). Each op accumulates into a part slot;
# the gate matmul's stacked wg1x2 sums the two partition halves.
GAP_TOP = [(0, QC, 0), (QC, 2 * QC + 2, 1)]
GAP_BOT = [(2 * QC, 3 * QC, 0), (3 * QC, FLAT, 1)]
QH = QC // 2
GAP_BOT4 = [
    (2 * QC, 2 * QC + QH, 0),
    (2 * QC + QH, 3 * QC, 1),
    (3 * QC, 3 * QC + QH, 2),
    (3 * QC + QH, FLAT, 3),
]


def _emit_gap_op(nc, pools, XX, part, win, is_bot, eng):
    a, b, slot = win
    lo, hi = (64, 128) if is_bot else (0, 64)
    src = XX[lo:hi, a:b]
    dst = pools["scrS" if eng == "act" else "scrD"][lo:hi]
    acc = part[lo:hi, slot : slot + 1]
    if eng == "act":
        return nc.scalar.activation(
            dst[:, 0 : b - a], src, ACTF.Copy, accum_out=acc
        )
    return nc.vector.tensor_scalar(
        dst[:, 0 : b - a], src, 0.0, 0.0, OP.add, OP.add, accum_out=acc
    )


def _emit_gate(nc, pools, s, pooled, consts, h_ext):
    """Gate MLP + softmax + top-2 for one sample (all f32).

    exp-without-max-sub (logits are small); folds the top-2 mask and
    renormalization: w8 = (u>=m2)*u / (sum((u>=m2)*u) + sum(u)*1e-8).
    Returns (wb_sb [128,E] f32 per-partition probs, b_comb [C,1]).
    """
    f = pools
    g = f["gate"]
    wg1x2_sb, bg1_sb, wg2_sb, bexp_sb, ones = consts
    n = lambda base: f"{base}{s}"

    h_ps = f["gpsum"].tile([GH, 1], F32, tag="cps", name=n("h_ps"))
    nc.tensor.matmul(h_ps[:], lhsT=wg1x2_sb, rhs=pooled[:], start=True, stop=True)
    nc.vector.tensor_scalar(h_ext[0:GH, :], h_ps[:], bg1_sb, 0.0, OP.add, OP.max)

    lg_ps = f["gpsum"].tile([1, E], F32, tag="cps", name=n("lg_ps"))
    nc.tensor.matmul(lg_ps[:], lhsT=h_ext[:], rhs=wg2_sb, start=True, stop=True)

    u = g.tile([1, E], F32, tag="u", name=n("u"))
    nc.scalar.activation(u[:], lg_ps[:], ACTF.Exp)
    usum = g.tile([1, 1], F32, tag="usum", name=n("usum"))
    nc.vector.tensor_reduce(usum[:], u[:], axis=AX.X, op=OP.add)
    m1p = g.tile([1, 1], F32, tag="m1p", name=n("m1p"))
    nc.vector.tensor_reduce(m1p[:], u[:], axis=AX.X, op=OP.max)
    pm = g.tile([1, E], F32, tag="pm", name=n("pm"))
    nc.vector.scalar_tensor_tensor(pm[:], u[:], m1p[:], u[:], op0=OP.is_lt, op1=OP.mult)
    m2 = g.tile([1, 1], F32, tag="m2", name=n("m2"))
    nc.vector.tensor_reduce(m2[:], pm[:], axis=AX.X, op=OP.max)
    spv = g.tile([1, E], F32, tag="spv", name=n("spv"))
    nc.vector.scalar_tensor_tensor(spv[:], u[:], m2[:], u[:], op0=OP.is_ge, op1=OP.mult)
    dsum = g.tile([1, 1], F32, tag="dsum", name=n("dsum"))
    nc.vector.tensor_reduce(dsum[:], spv[:], axis=AX.X, op=OP.add)
    dd = g.tile([1, 1], F32, tag="dd", name=n("dd"))
    nc.vector.scalar_tensor_tensor(dd[:], usum[:], 1e-8, dsum[:], op0=OP.mult, op1=OP.add)
    rr = g.tile([1, 1], F32, tag="rr", name=n("rr"))
    nc.vector.reciprocal(rr[:], dd[:])
    w8 = g.tile([1, E], F32, tag="w8", name=n("w8"))
    nc.vector.tensor_scalar_mul(w8[:], spv[:], rr[:])

    # broadcast w8 down all 128 partitions, then stage to SBUF for MACs
    wb_ps = f["gpsum"].tile([128, E], F32, tag="cps", name=n("wb_ps"))
    nc.tensor.matmul(wb_ps[:], lhsT=ones[:], rhs=w8[:], start=True, stop=True)
    wb_sb = g.tile([128, E], F32, tag="wb_sb", name=n("wb_sb"))
    nc.vector.tensor_copy(wb_sb[:], wb_ps[:])

    # combined bias: b_comb = b_exp^T @ w8^T
    w8c_ps = f["gpsum"].tile([E, 1], F32, tag="cps", name=n("w8c_ps"))
    nc.tensor.matmul(w8c_ps[:], lhsT=w8[:], rhs=ones[:, 0:1], start=True, stop=True)
    w8col = g.tile([E, 1], F32, tag="w8col", name=n("w8col"))
    nc.vector.tensor_copy(w8col[:], w8c_ps[:])
    bc_ps = f["gpsum"].tile([C, 1], F32, tag="cps", name=n("bc_ps"))
    nc.tensor.matmul(bc_ps[:], lhsT=bexp_sb, rhs=w8col[:], start=True, stop=True)
    b_comb = g.tile([C, 1], F32, tag="b_comb", name=n("b_comb"))
    nc.vector.tensor_copy(b_comb[:], bc_ps[:])
    return wb_sb, b_comb


def _emit_mac_pe(nc, pools, s, wb_sb, wpsA_sb, wpsB_sb, ident_sb):
    """wcomb = sum_e p_e wps_e on the PE: 8 accumulating matmuls with
    lhsT = p_e*I (built by ACT from the shipped identity with the
    per-partition probability as activation scale). Residual identity is
    pre-folded into every expert's center-tap B-half on the host."""
    f = pools
    pI = f["wcomb"].tile([128, E, 128], BF16, tag="pI", name=f"pI{s}")
    for e in range(E):
        nc.scalar.activation(
            pI[:, e, :], ident_sb[:], ACTF.Copy, scale=wb_sb[:, e : e + 1]
        )
    wcps = f["gpsum"].tile([128, 384], F32, tag="cps", name=f"wcps{s}")
    for e in range(E):
        src = wpsA_sb[:, e] if e < 4 else wpsB_sb[:, e - 4]
        nc.tensor.matmul(
            wcps[:],
            lhsT=pI[:, e, :],
            rhs=src.rearrange("p a b -> p (a b)"),
            start=(e == 0),
            stop=(e == E - 1),
        )
    wcombr = f["wcomb"].tile([128, 3, 128], BF16, tag="wcombr", name=f"wcombr{s}")
    nc.scalar.activation(
        wcombr[:].rearrange("p a b -> p (a b)"), wcps[:], ACTF.Copy
    )
    return wcombr


def _emit_pair(nc, pools, s, p, XX, wcombr, b_comb, ob, ocol, gps):
    """Conv for pair p: 6 matmuls (dy-major, N=ncol+1 so the stage's +1
    col realignment only reads written psum) into a 2-bank PSUM tile.
    ACT stages the B half with b_comb as activation bias; the combine is
    then obv = psA + stB (DVE tensor_tensor, or gpsimd via an extra ACT
    stage of the A half — gpsimd has no PSUM access)."""
    f = pools
    r0 = 6 * p
    last = p == NPAIR - 1
    nt = 1 if last else 2      # psum banks (3-row tiles) in this pair
    nr = 2 if last else 6      # rows
    ps = f["cpsum"].tile([128, 2, 512], F32, tag="cps", name=f"cps{s}_{p}")
    ncol = (nr // nt) * WP
    trows = nr // nt
    for dyi in range(3):
        for t in range(nt):
            ra = r0 + t * trows + dyi
            nc.tensor.matmul(
                ps[:, t, 0 : ncol + 1],
                lhsT=wcombr[:, dyi, :],
                rhs=XX[:, ra * WP : ra * WP + ncol + 1],
                start=(dyi == 0),
                stop=(dyi == 2),
            )
    obv = ob[:, ocol : ocol + nt * ncol].rearrange("p (t c) -> p t c", c=ncol)
    stB = f["stage"].tile([64, 2, 390], BF16, tag="stB", name=f"stB{s}_{p}")
    nc.scalar.activation(stB[:, 0:nt, 0:ncol], ps[64:128, 0:nt, 1 : ncol + 1], ACTF.Copy)
    if gps is not None:
        # gpsimd combine (TensorTensor only there, no PSUM access): ACT
        # stages the A half too; bias comes from the per-sample broadcast
        # tile in a second add
        stA = f["stage"].tile([64, 2, 390], BF16, tag="stA", name=f"stA{s}_{p}")
        nc.scalar.activation(stA[:, 0:nt, 0:ncol], ps[0:64, 0:nt, 0:ncol], ACTF.Copy)
        nc.gpsimd.tensor_tensor(
            obv, stA[:, 0:nt, 0:ncol], stB[:, 0:nt, 0:ncol], op=OP.add
        )
        return nc.gpsimd.tensor_tensor(obv, obv, gps[:, 0:nt, 0:ncol], op=OP.add)
    return nc.vector.scalar_tensor_tensor(
        obv,
        ps[0:64, 0:nt, 0:ncol],
        b_comb[:],
        stB[:, 0:nt, 0:ncol],
        op0=OP.add,
        op1=OP.add,
    )


def build_program():
    if "nc" in _cache:
        return _cache["nc"]
    nc = bacc.Bacc("TRN2", target_bir_lowering=False, debug=False, enable_asserts=False)
    xs_ap = nc.dram_tensor("xs", [SPB, 128, FLAT], BF16, kind="ExternalInput").ap()
    wpsA_d = nc.dram_tensor("wpsA", [128, E // 2, 3, 128], BF16, kind="ExternalInput").ap()
    wpsB_d = nc.dram_tensor("wpsB", [128, E // 2, 3, 128], BF16, kind="ExternalInput").ap()
    ident_d = nc.dram_tensor("ident", [128, 128], BF16, kind="ExternalInput").ap()
    ident_d = nc.dram_tensor("ident", [128, 128], BF16, kind="ExternalInput").ap()
    gconst_d = nc.dram_tensor("gconst", [128, 90], F32, kind="ExternalInput").ap()
    out_ap = nc.dram_tensor("out", [SPB, C, H * WP], BF16, kind="ExternalOutput").ap()

    with tile.TileContext(nc) as tc, ExitStack() as ctx:
        pools = {
            "const": ctx.enter_context(tc.tile_pool(name="const", bufs=1)),
            "xx": ctx.enter_context(tc.tile_pool(name="xx", bufs=SPB)),
            "gate": ctx.enter_context(tc.tile_pool(name="gate", bufs=2)),
            "wcomb": ctx.enter_context(tc.tile_pool(name="wcomb", bufs=2)),
            "stage": ctx.enter_context(tc.tile_pool(name="stage", bufs=6)),
            "cpsum": ctx.enter_context(tc.tile_pool(name="cpsum", bufs=3, space="PSUM")),
            "gpsum": ctx.enter_context(tc.tile_pool(name="gpsum", bufs=2, space="PSUM")),
        }
        cp = pools["const"]
        # +4 zeroed pad cols so the tail tile's widened matmul read stays
        # in bounds
        XX0 = pools["xx"].tile([128, FLAT + 4], BF16, tag="XX", name="XX0")
        XX1 = pools["xx"].tile([128, FLAT + 4], BF16, tag="XX", name="XX1")
        nc.vector.memset(XX0[:, FLAT : FLAT + 4], 0.0)
        nc.vector.memset(XX1[:, FLAT : FLAT + 4], 0.0)
        gconst_sb = cp.tile([128, 90], F32)
        ones = cp.tile([1, 128], F32)
        nc.gpsimd.memset(ones[:], 1.0)
        wpsA_sb = cp.tile([128, E // 2, 3, 128], BF16)
        wpsB_sb = cp.tile([128, E // 2, 3, 128], BF16)
        ident_sb = cp.tile([128, 128], BF16)
        ident_sb = cp.tile([128, 128], BF16)
        pools["scrD"] = cp.tile([128, QC + 2], BF16, name="scrD")
        pools["scrS"] = cp.tile([128, QC + 2], BF16, name="scrS")

        # ---- loads + prologue compute, interleaved so each consumer's
        # queue drain covers only the transfers it actually needs (a
        # consumer emitted after later triggers on a lane waits for ALL
        # of them - this drain effect, not bandwidth, dominated the v4/v6
        # prologues) ----
        C3A = 3 * QC + 2113
        nc.scalar.dma_start(wpsA_sb[:], wpsA_d[:])       # warmup needs it
        nc.scalar.dma_start(ident_sb[:], ident_d[:])

        # PE warm-up: no-DMA scratch matmuls start ~2us (HAM to 8/8),
        # then chunk-gated batches self-time the warm window to the load
        warm_sc = cp.tile([128, 384], BF16, name="warm_sc")
        nc.gpsimd.memset(warm_sc[:], 0.25)
        warm_ps = pools["gpsum"].tile([128, 384], F32, tag="cps", name="warm_ps")

        def emit_warm(n, rhs):
            for _ in range(n):
                nc.tensor.matmul(
                    warm_ps[:], lhsT=warm_sc[:, 0:128], rhs=rhs,
                    start=True, stop=True,
                )

        emit_warm(30, warm_sc[:])

        part0 = pools["gate"].tile([128, 2], F32, tag="part", name="part0")
        h_ext0 = pools["gate"].tile([GH + 1, 1], F32, tag="h_ext", name="h_ext0")
        h_ext1 = pools["gate"].tile([GH + 1, 1], F32, tag="h_ext", name="h_ext1")

        nc.sync.dma_start(XX0[:, 0:QC], xs_ap[0, :, 0:QC])
        nc.sync.dma_start(h_ext0[GH : GH + 1, 0:1], ones[0:1, 0:1])
        nc.sync.dma_start(h_ext1[GH : GH + 1, 0:1], ones[0:1, 0:1])
        w1i = _emit_gap_op(nc, pools, XX0, part0, GAP_TOP[0], is_bot=False, eng="act")
        emit_warm(8, XX0[:, 0:384])

        nc.gpsimd.dma_start(gconst_sb[:], gconst_d[:])
        nc.gpsimd.dma_start(XX0[:, QC : 2 * QC], xs_ap[0, :, QC : 2 * QC])
        nc.scalar.dma_start(XX0[:, 2 * QC : 3 * QC], xs_ap[0, :, 2 * QC : 3 * QC])
        w2i = _emit_gap_op(nc, pools, XX0, part0, GAP_TOP[1], is_bot=False, eng="dve")
        w3i = _emit_gap_op(nc, pools, XX0, part0, GAP_BOT[0], is_bot=True, eng="dve")
        emit_warm(8, XX0[:, QC : QC + 384])
        emit_warm(8, XX0[:, 2 * QC : 2 * QC + 384])

        nc.gpsimd.dma_start(XX0[:, C3A:FLAT], xs_ap[0, :, C3A:FLAT])
        nc.sync.dma_start(XX0[:, 3 * QC : C3A], xs_ap[0, :, 3 * QC : C3A])
        w4i = _emit_gap_op(nc, pools, XX0, part0, GAP_BOT[1], is_bot=True, eng="act")
        emit_warm(8, XX0[:, 3 * QC : 3 * QC + 384])

        nc.scalar.dma_start(wpsB_sb[:], wpsB_d[:])

        wg1x2_sb = gconst_sb[:, 0:16]
        bg1_sb = gconst_sb[0:16, 16:17]
        wg2_sb = gconst_sb[0:17, 17:25]
        bexp_sb = gconst_sb[0:8, 25:89]
        consts = (wg1x2_sb, bg1_sb, wg2_sb, bexp_sb, ones)

        pooled0 = pools["gate"].tile([128, 1], F32, tag="pooled", name="pooled0")
        nc.vector.tensor_reduce(pooled0, part0[:], axis=AX.X, op=OP.add)
        wb0, bcomb0 = _emit_gate(nc, pools, 0, pooled0, consts, h_ext0)
        wcombr0 = _emit_mac_pe(nc, pools, 0, wb0, wpsA_sb, wpsB_sb, ident_sb)
        zb = cp.tile([64, 2, 390], BF16, name="zb")
        nc.gpsimd.memset(zb[:], 0.0)
        bB0 = pools["gate"].tile([64, 2, 390], BF16, tag="bB", name="bB0")
        nc.vector.scalar_tensor_tensor(
            bB0[:], zb[:], bcomb0[:], zb[:], op0=OP.add, op1=OP.add
        )

        # ---- s1 x loads: each trigger HARD-pinned after an s0 GAP op so
        # no s0 consumer's queue drain can end up covering s1 transfers
        # (the scheduler otherwise floats these triggers early) ----
        def pin_trig(ti, gi):
            tile.add_dep_helper(
                ti.ins, gi.ins, sync=True,
                reason="s1 load held until s0 GAP consumed its lane",
            )

        pin_trig(nc.gpsimd.dma_start(XX1[:, 0:QC], xs_ap[1, :, 0:QC]), w1i)
        pin_trig(nc.sync.dma_start(XX1[:, QC : 2 * QC], xs_ap[1, :, QC : 2 * QC]), w2i)
        pin_trig(
            nc.scalar.dma_start(XX1[:, 2 * QC : 3 * QC], xs_ap[1, :, 2 * QC : 3 * QC]),
            w3i,
        )
        pin_trig(nc.sync.dma_start(XX1[:, 3 * QC : C3A], xs_ap[1, :, 3 * QC : C3A]), w4i)
        pin_trig(nc.gpsimd.dma_start(XX1[:, C3A:FLAT], xs_ap[1, :, C3A:FLAT]), w4i)

        part1 = pools["gate"].tile([128, 4], F32, tag="part", name="part1")
        nc.gpsimd.memset(part1[0:64, 2:4], 0.0)
        s1_state = {}

        def s1_hook(p, comb):
            def pin(gi):
                tile.add_dep_helper(
                    gi.ins, comb.ins, sync=False,
                    reason="s1 prep slotted after this pair's combine",
                )
            if p == 4:
                pin(_emit_gap_op(nc, pools, XX1, part1, GAP_TOP[0], is_bot=False, eng="act"))
            elif p == 7:
                pin(_emit_gap_op(nc, pools, XX1, part1, GAP_TOP[1], is_bot=False, eng="dve"))
            elif p in (9, 11, 13, 14):
                k = {9: 0, 11: 1, 13: 2, 14: 3}[p]
                pin(_emit_gap_op(nc, pools, XX1, part1, GAP_BOT4[k], is_bot=True, eng="dve"))
            elif p == 15:
                pooled1 = pools["gate"].tile(
                    [128, 1], F32, tag="pooled", name="pooled1"
                )
                pin(nc.vector.tensor_reduce(pooled1, part1[:], axis=AX.X, op=OP.add))
                wb1, bcomb1 = _emit_gate(nc, pools, 1, pooled1, consts, h_ext1)
                s1_state["bcomb"] = bcomb1
                s1_state["wcombr"] = _emit_mac_pe(
                    nc, pools, 1, wb1, wpsA_sb, wpsB_sb, ident_sb
                )
                bB1 = pools["gate"].tile([64, 2, 390], BF16, tag="bB", name="bB1")
                nc.vector.scalar_tensor_tensor(
                    bB1[:], zb[:], bcomb1[:], zb[:], op0=OP.add, op1=OP.add
                )
                s1_state["bB"] = bB1

        # out batching: one [64, OBW] buffer per 24-row batch (batch 5 is
        # 8 rows); s0 batches drain on SP, s1 batches on gpsimd
        obstate = {0: [None, 0], 1: [None, 0]}

        bBmap = {}

        def emit_sample_pairs(s, XX, wcombr, bcomb, rng, hook=None):
            for p in rng:
                batch = min(p // 4, 5)
                ob, ocol = obstate[s]
                if ob is None:
                    ob = pools["stage"].tile(
                        [64, OBW], BF16, tag="ob", name=f"ob{s}_{batch}", bufs=3
                    )
                    obstate[s] = [ob, 0]
                    ocol = 0
                gps = bBmap.get(s) if (s == 0 and p in GPS_PAIRS_S0) else None
                comb = _emit_pair(nc, pools, s, p, XX, wcombr, bcomb, ob, ocol, gps)
                ocol += 780 if p < NPAIR - 1 else 260
                obstate[s][1] = ocol
                bcols = OBW if batch < 5 else 1040
                if ocol == bcols:
                    lane = nc.sync if s == 0 else nc.gpsimd
                    lane.dma_start(
                        out_ap[s, :, 24 * batch * WP : 24 * batch * WP + bcols],
                        ob[:, 0:bcols],
                    )
                    obstate[s] = [None, 0]
                if hook is not None:
                    hook(p, comb)

        bBmap[0] = bB0
        emit_sample_pairs(0, XX0, wcombr0, bcomb0, range(NPAIR), s1_hook)
        emit_sample_pairs(
            1, XX1, s1_state["wcombr"], s1_state["bcomb"], range(NPAIR)
        )

    nc.compile()
    _cache["nc"] = nc
    return nc


def host_prep(x, wg1, bg1, wg2, bg2, w_exp, b_exp):
    """Host-side layout prep + per-core sharding. Returns in_maps list."""
    x = np.asarray(x, dtype=np.float32)
    wg1 = np.asarray(wg1, dtype=np.float32)
    bg1 = np.asarray(bg1, dtype=np.float32)
    wg2 = np.asarray(wg2, dtype=np.float32)
    bg2 = np.asarray(bg2, dtype=np.float32)
    w_exp = np.asarray(w_exp, dtype=np.float32)
    b_exp = np.asarray(b_exp, dtype=np.float32)

    # x shipped as [B, 128, FLAT] bf16: rows 0:64 = zero-padded flat
    # image, rows 64:128 = the same shifted +2 elements (the conv's
    # bottom-half K copy) — both SBUF halves land in one full-rate DMA
    xpad = np.zeros((B, C, HP, WP), np.float32)
    xpad[:, :, 1 : H + 1, 1 : W + 1] = x
    flat = xpad.reshape(B, C, FLAT)
    xs = np.zeros((B, 128, FLAT), NPBF16)
    xs[:, 0:64] = flat.astype(NPBF16)
    xs[:, 64:128, 0 : FLAT - 2] = flat[:, :, 2:].astype(NPBF16)

    # wps [128, E, 3(dy), 128]: K top/bottom = taps dx 0/2 on M 0:64 (A),
    # center dx=1 on M 64:128 top (B, bottom zero). Residual identity is
    # folded into every expert's center tap (sum of probs is ~1).
    wt = np.transpose(w_exp, (2, 0, 3, 4, 1))  # [I, E, dy, dx, O]
    wps = np.zeros((128, E, 3, 128), np.float32)
    wps[0:64, :, :, 0:64] = wt[:, :, :, 0, :]
    wps[64:128, :, :, 0:64] = wt[:, :, :, 2, :]
    wps[0:64, :, :, 64:128] = wt[:, :, :, 1, :]
    ii = np.arange(64)
    wps[ii, :, 1, 64 + ii] += 1.0

    gconst = np.zeros((128, 90), np.float32)
    gconst[:, 0:16] = np.concatenate([wg1, wg1], axis=0) / (H * W)
    gconst[0:16, 16] = bg1
    gconst[0:16, 17:25] = wg2
    gconst[16, 17:25] = bg2
    gconst[0:8, 25:89] = b_exp

    shared = {
        "wpsA": np.ascontiguousarray(wps[:, 0:4]).astype(NPBF16),
        "wpsB": np.ascontiguousarray(wps[:, 4:8]).astype(NPBF16),
        "ident": np.eye(128, dtype=NPBF16),
        "gconst": gconst,
    }
    return [
        {"xs": np.ascontiguousarray(xs[SPB * k : SPB * (k + 1)]), **shared}
        for k in range(NCORES)
    ]


def _decode_out(o):
    """[C, H*WP] bf16 -> [C, H, W] f32 (strip the pad columns)."""
    return np.asarray(o, dtype=np.float32).reshape(C, H, WP)[:, :, 0:W]


def kernel(x, wg1, bg1, wg2, bg2, w_exp, b_exp):
    nc = build_program()
    in_maps = host_prep(x, wg1, bg1, wg2, bg2, w_exp, b_exp)
    res = run_bass_kernel_spmd(nc, in_maps, list(range(NCORES)))
    out = np.empty((B, C, H, W), np.float32)
    for k in range(NCORES):
        o = np.asarray(res.results[k]["out"])
        for s in range(SPB):
            out[SPB * k + s] = _decode_out(o[s])
    return out


# revision 30
# speedup vs baseline: 1.0782x; 1.0064x over previous
"""MoE block (top-2 routed 3x3 conv experts) Trainium2 Bass kernel — v6.

Data-parallel over batch, 2 samples per core on 8 cores. The conv is
linear in the kernel, so the top-2 expert kernels are combined with the
routing probabilities first (w_comb = sum_e p_e W_e + I, the +I folding
the residual into the center tap), then one 3x3 SAME conv per sample.
Conv-as-matmul: 6 N=391 matmuls per 6-row pair (dy-major), A-half taps
on psum partitions 0:64, center taps on 64:128 realigned +1 col by the
ACT stage, combined into [64, OBW] out batches.

v6 structure (v4 baseline 118.5us; v5 experiments: TTR crashes HW at
runtime, cross-base SBUF binary ops and 2-bank matmul outs are ISA
errors, gpsimd lacks TensorScalarPtr/TensorReduce):
- Loads x-first on all 3 DMA lanes (SP/gpsimd/ACT), weights slotted
  where they don't delay the GAP->gate->MAC critical path. Per-lane
  effective bandwidth is only ~90-130 GB/s, so s0's chunks are spread
  across all lanes and s1 streams behind them.
- PE warm-up block (~85 throwaway matmuls on wpsA) keeps the HAM clock
  gate at 8/8 through the prologue so real convs start at 2.4 GHz.
- MAC on the PE: wcomb_psum = sum_e (p_e*I)^T @ wps_e — 8 ACT ops build
  p_e*I from a shipped identity (per-partition scale), 8 accumulating
  matmuls, one ACT copy back to bf16. Replaces the serial DVE MAC chain
  (~7-9us/sample) with ~2us ACT + ~1.3us PE.
- b_comb folded into the B-half stage as the ACT activation bias; the
  combine is then a 2-operand tensor_tensor add. DVE does most pairs;
  4 late-s0 pairs go to gpsimd (tensor_tensor IS supported there) via a
  double ACT stage, relieving the DVE in the tight s0 phase.
- GAP: v4-style accumulating windows (ACT tops half / DVE the rest),
  s1's windows and gate emitted between s0 pairs as its chunks land.
"""
import os
import numpy as np
from contextlib import ExitStack

import ml_dtypes

import concourse.bass as bass
import concourse.tile as tile
from concourse import bacc, mybir
from concourse.bass_utils import run_bass_kernel_spmd
import concourse.bass_utils as _bu

if os.environ.get("KLDW") and not getattr(_bu, "_kldw_patched", False):
    _orig_run_command = _bu.run_command

    def _run_command(cmd, **kw):
        cmd = [
            c.replace("--enable-ldw-opt=false", "--enable-ldw-opt=true")
            if isinstance(c, str) else c
            for c in cmd
        ]
        return _orig_run_command(cmd, **kw)

    _bu.run_command = _run_command
    _bu._kldw_patched = True

F32 = mybir.dt.float32
BF16 = mybir.dt.bfloat16
AX = mybir.AxisListType
OP = mybir.AluOpType
ACTF = mybir.ActivationFunctionType

B, C, H, W, E, GH = 16, 64, 128, 128, 8, 16
NCORES = 8
SPB = B // NCORES          # samples per core
HP, WP = H + 2, W + 2      # 130
FLAT = HP * WP             # 16900
QC = FLAT // 4             # x-load chunk size (4225 flat elements)
OBW = 24 * WP              # out batch region width (3120)
NPAIR = 22                 # 21 six-row pairs + one trailing 2-row tile
WARM_MM = 85               # PE warm-up matmuls (span the prologue)
GPS_PAIRS_S0 = {10, 13, 16, 19}   # s0 pairs whose combine runs on gpsimd

NPBF16 = ml_dtypes.bfloat16

_cache = {}

# GAP windows over the flat layout (pad zeros included): top copy
# (partitions 0:64) covers flat[0:2QC+2), bottom copy (64:128, shifted
# +2) covers flat[2QC+2:FLAT). Each op accumulates into a part slot;
# the gate matmul's stacked wg1x2 sums the two partition halves.
GAP_TOP = [(0, QC, 0), (QC, 2 * QC + 2, 1)]
GAP_BOT = [(2 * QC, 3 * QC, 0), (3 * QC, FLAT, 1)]
QH = QC // 2
GAP_BOT4 = [
    (2 * QC, 2 * QC + QH, 0),
    (2 * QC + QH, 3 * QC, 1),
    (3 * QC, 3 * QC + QH, 2),
    (3 * QC + QH, FLAT, 3),
]


def _emit_gap_op(nc, pools, XX, part, win, is_bot, eng):
    a, b, slot = win
    lo, hi = (64, 128) if is_bot else (0, 64)
    src = XX[lo:hi, a:b]
    dst = pools["scrS" if eng == "act" else "scrD"][lo:hi]
    acc = part[lo:hi, slot : slot + 1]
    if eng == "act":
        return nc.scalar.activation(
            dst[:, 0 : b - a], src, ACTF.Copy, accum_out=acc
        )
    return nc.vector.tensor_scalar(
        dst[:, 0 : b - a], src, 0.0, 0.0, OP.add, OP.add, accum_out=acc
    )


def _emit_gate(nc, pools, s, pooled, consts, h_ext):
    """Gate MLP + softmax + top-2 for one sample (all f32).

    exp-without-max-sub (logits are small); folds the top-2 mask and
    renormalization: w8 = (u>=m2)*u / (sum((u>=m2)*u) + sum(u)*1e-8).
    Returns (wb_sb [128,E] f32 per-partition probs, b_comb [C,1]).
    """
    f = pools
    g = f["gate"]
    wg1x2_sb, bg1_sb, wg2_sb, bexp_sb, ones = consts
    n = lambda base: f"{base}{s}"

    h_ps = f["gpsum"].tile([GH, 1], F32, tag="cps", name=n("h_ps"))
    nc.tensor.matmul(h_ps[:], lhsT=wg1x2_sb, rhs=pooled[:], start=True, stop=True)
    nc.vector.tensor_scalar(h_ext[0:GH, :], h_ps[:], bg1_sb, 0.0, OP.add, OP.max)

    lg_ps = f["gpsum"].tile([1, E], F32, tag="cps", name=n("lg_ps"))
    nc.tensor.matmul(lg_ps[:], lhsT=h_ext[:], rhs=wg2_sb, start=True, stop=True)

    u = g.tile([1, E], F32, tag="u", name=n("u"))
    nc.scalar.activation(u[:], lg_ps[:], ACTF.Exp)
    usum = g.tile([1, 1], F32, tag="usum", name=n("usum"))
    nc.vector.tensor_reduce(usum[:], u[:], axis=AX.X, op=OP.add)
    m1p = g.tile([1, 1], F32, tag="m1p", name=n("m1p"))
    nc.vector.tensor_reduce(m1p[:], u[:], axis=AX.X, op=OP.max)
    pm = g.tile([1, E], F32, tag="pm", name=n("pm"))
    nc.vector.scalar_tensor_tensor(pm[:], u[:], m1p[:], u[:], op0=OP.is_lt, op1=OP.mult)
    m2 = g.tile([1, 1], F32, tag="m2", name=n("m2"))
    nc.vector.tensor_reduce(m2[:], pm[:], axis=AX.X, op=OP.max)
    spv = g.tile([1, E], F32, tag="spv", name=n("spv"))
    nc.vector.scalar_tensor_tensor(spv[:], u[:], m2[:], u[:], op0=OP.is_ge, op1=OP.mult)
    dsum = g.tile([1, 1], F32, tag="dsum", name=n("dsum"))
    nc.vector.tensor_reduce(dsum[:], spv[:], axis=AX.X, op=OP.add)
    dd = g.tile([1, 1], F32, tag="dd", name=n("dd"))
    nc.vector.scalar_tensor_tensor(dd[:], usum[:], 1e-8, dsum[:], op0=OP.mult, op1=OP.add)
    rr = g.tile([1, 1], F32, tag="rr", name=n("rr"))
    nc.vector.reciprocal(rr[:], dd[:])
    w8 = g.tile([1, E], F32, tag="w8", name=n("w8"))
    nc.vector.tensor_scalar_mul(w8[:], spv[:], rr[:])

    # broadcast w8 down all 128 partitions, then stage to SBUF for MACs
    wb_ps = f["gpsum"].tile([128, E], F32, tag="cps", name=n("wb_ps"))
    nc.tensor.matmul(wb_ps[:], lhsT=ones[:], rhs=w8[:], start=True, stop=True)
    wb_sb = g.tile([128, E], F32, tag="wb_sb", name=n("wb_sb"))
    nc.vector.tensor_copy(wb_sb[:], wb_ps[:])

    # combined bias: b_comb = b_exp^T @ w8^T
    w8c_ps = f["gpsum"].tile([E, 1], F32, tag="cps", name=n("w8c_ps"))
    nc.tensor.matmul(w8c_ps[:], lhsT=w8[:], rhs=ones[:, 0:1], start=True, stop=True)
    w8col = g.tile([E, 1], F32, tag="w8col", name=n("w8col"))
    nc.vector.tensor_copy(w8col[:], w8c_ps[:])
    bc_ps = f["gpsum"].tile([C, 1], F32, tag="cps", name=n("bc_ps"))
    nc.tensor.matmul(bc_ps[:], lhsT=bexp_sb, rhs=w8col[:], start=True, stop=True)
    b_comb = g.tile([C, 1], F32, tag="b_comb", name=n("b_comb"))
    nc.vector.tensor_copy(b_comb[:], bc_ps[:])
    return wb_sb, b_comb


def _emit_mac_pe(nc, pools, s, wb_sb, wpsA_sb, wpsB_sb, ident_sb):
    """wcomb = sum_e p_e wps_e on the PE: 8 accumulating matmuls with
    lhsT = p_e*I (built by ACT from the shipped identity with the
    per-partition probability as activation scale). Residual identity is
    pre-folded into every expert's center-tap B-half on the host."""
    f = pools
    pI = f["wcomb"].tile([128, E, 128], BF16, tag="pI", name=f"pI{s}")
    for e in range(E):
        nc.scalar.activation(
            pI[:, e, :], ident_sb[:], ACTF.Copy, scale=wb_sb[:, e : e + 1]
        )
    wcps = f["gpsum"].tile([128, 384], F32, tag="cps", name=f"wcps{s}")
    for e in range(E):
        src = wpsA_sb[:, e] if e < 4 else wpsB_sb[:, e - 4]
        nc.tensor.matmul(
            wcps[:],
            lhsT=pI[:, e, :],
            rhs=src.rearrange("p a b -> p (a b)"),
            start=(e == 0),
            stop=(e == E - 1),
        )
    wcombr = f["wcomb"].tile([128, 3, 128], BF16, tag="wcombr", name=f"wcombr{s}")
    nc.scalar.activation(
        wcombr[:].rearrange("p a b -> p (a b)"), wcps[:], ACTF.Copy
    )
    return wcombr


def _emit_pair(nc, pools, s, p, XX, wcombr, b_comb, ob, ocol, gps):
    """Conv for pair p: 6 matmuls (dy-major, N=ncol+1 so the stage's +1
    col realignment only reads written psum) into a 2-bank PSUM tile.
    ACT stages the B half with b_comb as activation bias; the combine is
    then obv = psA + stB (DVE tensor_tensor, or gpsimd via an extra ACT
    stage of the A half — gpsimd has no PSUM access)."""
    f = pools
    r0 = 6 * p
    last = p == NPAIR - 1
    nt = 1 if last else 2      # psum banks (3-row tiles) in this pair
    nr = 2 if last else 6      # rows
    ps = f["cpsum"].tile([128, 2, 512], F32, tag="cps", name=f"cps{s}_{p}")
    ncol = (nr // nt) * WP
    trows = nr // nt
    for dyi in range(3):
        for t in range(nt):
            ra = r0 + t * trows + dyi
            nc.tensor.matmul(
                ps[:, t, 0 : ncol + 1],
                lhsT=wcombr[:, dyi, :],
                rhs=XX[:, ra * WP : ra * WP + ncol + 1],
                start=(dyi == 0),
                stop=(dyi == 2),
            )
    obv = ob[:, ocol : ocol + nt * ncol].rearrange("p (t c) -> p t c", c=ncol)
    stB = f["stage"].tile([64, 2, 390], BF16, tag="stB", name=f"stB{s}_{p}")
    nc.scalar.activation(stB[:, 0:nt, 0:ncol], ps[64:128, 0:nt, 1 : ncol + 1], ACTF.Copy)
    if gps is not None:
        # gpsimd combine (TensorTensor only there, no PSUM access): ACT
        # stages the A half too; bias comes from the per-sample broadcast
        # tile in a second add
        stA = f["stage"].tile([64, 2, 390], BF16, tag="stA", name=f"stA{s}_{p}")
        nc.scalar.activation(stA[:, 0:nt, 0:ncol], ps[0:64, 0:nt, 0:ncol], ACTF.Copy)
        nc.gpsimd.tensor_tensor(
            obv, stA[:, 0:nt, 0:ncol], stB[:, 0:nt, 0:ncol], op=OP.add
        )
        return nc.gpsimd.tensor_tensor(obv, obv, gps[:, 0:nt, 0:ncol], op=OP.add)
    return nc.vector.scalar_tensor_tensor(
        obv,
        ps[0:64, 0:nt, 0:ncol],
        b_comb[:],
        stB[:, 0:nt, 0:ncol],
        op0=OP.add,
        op1=OP.add,
    )


def build_program():
    if "nc" in _cache:
        return _cache["nc"]
    nc = bacc.Bacc("TRN2", target_bir_lowering=False, debug=False, enable_asserts=False, dynamic_dma_scratch_size=65536)
    xs_ap = nc.dram_tensor("xs", [SPB, 128, FLAT], BF16, kind="ExternalInput").ap()
    wpsA_d = nc.dram_tensor("wpsA", [128, E // 2, 3, 128], BF16, kind="ExternalInput").ap()
    wpsB_d = nc.dram_tensor("wpsB", [128, E // 2, 3, 128], BF16, kind="ExternalInput").ap()
    ident_d = nc.dram_tensor("ident", [128, 128], BF16, kind="ExternalInput").ap()
    ident_d = nc.dram_tensor("ident", [128, 128], BF16, kind="ExternalInput").ap()
    gconst_d = nc.dram_tensor("gconst", [128, 90], F32, kind="ExternalInput").ap()
    out_ap = nc.dram_tensor("out", [SPB, C, H * WP], BF16, kind="ExternalOutput").ap()

    with tile.TileContext(nc) as tc, ExitStack() as ctx:
        pools = {
            "const": ctx.enter_context(tc.tile_pool(name="const", bufs=1)),
            "xx": ctx.enter_context(tc.tile_pool(name="xx", bufs=SPB)),
            "gate": ctx.enter_context(tc.tile_pool(name="gate", bufs=2)),
            "wcomb": ctx.enter_context(tc.tile_pool(name="wcomb", bufs=2)),
            "stage": ctx.enter_context(tc.tile_pool(name="stage", bufs=6)),
            "cpsum": ctx.enter_context(tc.tile_pool(name="cpsum", bufs=3, space="PSUM")),
            "gpsum": ctx.enter_context(tc.tile_pool(name="gpsum", bufs=2, space="PSUM")),
        }
        cp = pools["const"]
        # +4 zeroed pad cols so the tail tile's widened matmul read stays
        # in bounds
        XX0 = pools["xx"].tile([128, FLAT + 4], BF16, tag="XX", name="XX0")
        XX1 = pools["xx"].tile([128, FLAT + 4], BF16, tag="XX", name="XX1")
        nc.vector.memset(XX0[:, FLAT : FLAT + 4], 0.0)
        nc.vector.memset(XX1[:, FLAT : FLAT + 4], 0.0)
        gconst_sb = cp.tile([128, 90], F32)
        ones = cp.tile([1, 128], F32)
        nc.gpsimd.memset(ones[:], 1.0)
        wpsA_sb = cp.tile([128, E // 2, 3, 128], BF16)
        wpsB_sb = cp.tile([128, E // 2, 3, 128], BF16)
        ident_sb = cp.tile([128, 128], BF16)
        ident_sb = cp.tile([128, 128], BF16)
        pools["scrD"] = cp.tile([128, QC + 2], BF16, name="scrD")
        pools["scrS"] = cp.tile([128, QC + 2], BF16, name="scrS")

        # ---- loads + prologue compute, interleaved so each consumer's
        # queue drain covers only the transfers it actually needs (a
        # consumer emitted after later triggers on a lane waits for ALL
        # of them - this drain effect, not bandwidth, dominated the v4/v6
        # prologues) ----
        C3A = 3 * QC + 2113
        nc.scalar.dma_start(wpsA_sb[:], wpsA_d[:])       # warmup needs it
        nc.scalar.dma_start(ident_sb[:], ident_d[:])

        # PE warm-up: no-DMA scratch matmuls start ~2us (HAM to 8/8),
        # then chunk-gated batches self-time the warm window to the load
        warm_sc = cp.tile([128, 384], BF16, name="warm_sc")
        nc.gpsimd.memset(warm_sc[:], 0.25)
        warm_ps = pools["gpsum"].tile([128, 384], F32, tag="cps", name="warm_ps")

        def emit_warm(n, rhs):
            for _ in range(n):
                nc.tensor.matmul(
                    warm_ps[:], lhsT=warm_sc[:, 0:128], rhs=rhs,
                    start=True, stop=True,
                )

        emit_warm(30, warm_sc[:])

        part0 = pools["gate"].tile([128, 2], F32, tag="part", name="part0")
        h_ext0 = pools["gate"].tile([GH + 1, 1], F32, tag="h_ext", name="h_ext0")
        h_ext1 = pools["gate"].tile([GH + 1, 1], F32, tag="h_ext", name="h_ext1")

        nc.sync.dma_start(XX0[:, 0:QC], xs_ap[0, :, 0:QC])
        nc.sync.dma_start(h_ext0[GH : GH + 1, 0:1], ones[0:1, 0:1])
        nc.sync.dma_start(h_ext1[GH : GH + 1, 0:1], ones[0:1, 0:1])
        w1i = _emit_gap_op(nc, pools, XX0, part0, GAP_TOP[0], is_bot=False, eng="act")
        emit_warm(8, XX0[:, 0:384])

        nc.gpsimd.dma_start(gconst_sb[:], gconst_d[:])
        nc.gpsimd.dma_start(XX0[:, QC : 2 * QC], xs_ap[0, :, QC : 2 * QC])
        nc.scalar.dma_start(XX0[:, 2 * QC : 3 * QC], xs_ap[0, :, 2 * QC : 3 * QC])
        w2i = _emit_gap_op(nc, pools, XX0, part0, GAP_TOP[1], is_bot=False, eng="dve")
        w3i = _emit_gap_op(nc, pools, XX0, part0, GAP_BOT[0], is_bot=True, eng="dve")
        emit_warm(8, XX0[:, QC : QC + 384])
        emit_warm(8, XX0[:, 2 * QC : 2 * QC + 384])

        nc.gpsimd.dma_start(XX0[:, C3A:FLAT], xs_ap[0, :, C3A:FLAT])
        nc.sync.dma_start(XX0[:, 3 * QC : C3A], xs_ap[0, :, 3 * QC : C3A])
        w4i = _emit_gap_op(nc, pools, XX0, part0, GAP_BOT[1], is_bot=True, eng="act")
        emit_warm(8, XX0[:, 3 * QC : 3 * QC + 384])

        nc.scalar.dma_start(wpsB_sb[:], wpsB_d[:])

        wg1x2_sb = gconst_sb[:, 0:16]
        bg1_sb = gconst_sb[0:16, 16:17]
        wg2_sb = gconst_sb[0:17, 17:25]
        bexp_sb = gconst_sb[0:8, 25:89]
        consts = (wg1x2_sb, bg1_sb, wg2_sb, bexp_sb, ones)

        pooled0 = pools["gate"].tile([128, 1], F32, tag="pooled", name="pooled0")
        nc.vector.tensor_reduce(pooled0, part0[:], axis=AX.X, op=OP.add)
        wb0, bcomb0 = _emit_gate(nc, pools, 0, pooled0, consts, h_ext0)
        wcombr0 = _emit_mac_pe(nc, pools, 0, wb0, wpsA_sb, wpsB_sb, ident_sb)
        zb = cp.tile([64, 2, 390], BF16, name="zb")
        nc.gpsimd.memset(zb[:], 0.0)
        bB0 = pools["gate"].tile([64, 2, 390], BF16, tag="bB", name="bB0")
        nc.vector.scalar_tensor_tensor(
            bB0[:], zb[:], bcomb0[:], zb[:], op0=OP.add, op1=OP.add
        )

        # ---- s1 x loads: each trigger HARD-pinned after an s0 GAP op so
        # no s0 consumer's queue drain can end up covering s1 transfers
        # (the scheduler otherwise floats these triggers early) ----
        def pin_trig(ti, gi):
            tile.add_dep_helper(
                ti.ins, gi.ins, sync=True,
                reason="s1 load held until s0 GAP consumed its lane",
            )

        pin_trig(nc.gpsimd.dma_start(XX1[:, 0:QC], xs_ap[1, :, 0:QC]), w1i)
        pin_trig(nc.sync.dma_start(XX1[:, QC : 2 * QC], xs_ap[1, :, QC : 2 * QC]), w2i)
        pin_trig(
            nc.scalar.dma_start(XX1[:, 2 * QC : 3 * QC], xs_ap[1, :, 2 * QC : 3 * QC]),
            w3i,
        )
        pin_trig(nc.sync.dma_start(XX1[:, 3 * QC : C3A], xs_ap[1, :, 3 * QC : C3A]), w4i)
        pin_trig(nc.gpsimd.dma_start(XX1[:, C3A:FLAT], xs_ap[1, :, C3A:FLAT]), w4i)

        part1 = pools["gate"].tile([128, 4], F32, tag="part", name="part1")
        nc.gpsimd.memset(part1[0:64, 2:4], 0.0)
        s1_state = {}

        def s1_hook(p, comb):
            def pin(gi):
                tile.add_dep_helper(
                    gi.ins, comb.ins, sync=False,
                    reason="s1 prep slotted after this pair's combine",
                )
            if p == 4:
                pin(_emit_gap_op(nc, pools, XX1, part1, GAP_TOP[0], is_bot=False, eng="act"))
            elif p == 7:
                pin(_emit_gap_op(nc, pools, XX1, part1, GAP_TOP[1], is_bot=False, eng="dve"))
            elif p in (9, 11, 13, 14):
                k = {9: 0, 11: 1, 13: 2, 14: 3}[p]
                pin(_emit_gap_op(nc, pools, XX1, part1, GAP_BOT4[k], is_bot=True, eng="dve"))
            elif p == 15:
                pooled1 = pools["gate"].tile(
                    [128, 1], F32, tag="pooled", name="pooled1"
                )
                pin(nc.vector.tensor_reduce(pooled1, part1[:], axis=AX.X, op=OP.add))
                wb1, bcomb1 = _emit_gate(nc, pools, 1, pooled1, consts, h_ext1)
                s1_state["bcomb"] = bcomb1
                s1_state["wcombr"] = _emit_mac_pe(
                    nc, pools, 1, wb1, wpsA_sb, wpsB_sb, ident_sb
                )
                bB1 = pools["gate"].tile([64, 2, 390], BF16, tag="bB", name="bB1")
                nc.vector.scalar_tensor_tensor(
                    bB1[:], zb[:], bcomb1[:], zb[:], op0=OP.add, op1=OP.add
                )
                s1_state["bB"] = bB1

        # out batching: one [64, OBW] buffer per 24-row batch (batch 5 is
        # 8 rows); s0 batches drain on SP, s1 batches on gpsimd
        obstate = {0: [None, 0], 1: [None, 0]}

        bBmap = {}

        def emit_sample_pairs(s, XX, wcombr, bcomb, rng, hook=None):
            for p in rng:
                batch = min(p // 4, 5)
                ob, ocol = obstate[s]
                if ob is None:
                    ob = pools["stage"].tile(
                        [64, OBW], BF16, tag="ob", name=f"ob{s}_{batch}", bufs=3
                    )
                    obstate[s] = [ob, 0]
                    ocol = 0
                gps = bBmap.get(s) if (s == 0 and p in GPS_PAIRS_S0) else None
                comb = _emit_pair(nc, pools, s, p, XX, wcombr, bcomb, ob, ocol, gps)
                ocol += 780 if p < NPAIR - 1 else 260
                obstate[s][1] = ocol
                bcols = OBW if batch < 5 else 1040
                if ocol == bcols:
                    lane = nc.sync if s == 0 else nc.gpsimd
                    lane.dma_start(
                        out_ap[s, :, 24 * batch * WP : 24 * batch * WP + bcols],
                        ob[:, 0:bcols],
                    )
                    obstate[s] = [None, 0]
                if hook is not None:
                    hook(p, comb)

        bBmap[0] = bB0
        emit_sample_pairs(0, XX0, wcombr0, bcomb0, range(NPAIR), s1_hook)
        emit_sample_pairs(
            1, XX1, s1_state["wcombr"], s1_state["bcomb"], range(NPAIR)
        )

    nc.compile()
    _cache["nc"] = nc
    return nc


def host_prep(x, wg1, bg1, wg2, bg2, w_exp, b_exp):
    """Host-side layout prep + per-core sharding. Returns in_maps list."""
    x = np.asarray(x, dtype=np.float32)
    wg1 = np.asarray(wg1, dtype=np.float32)
    bg1 = np.asarray(bg1, dtype=np.float32)
    wg2 = np.asarray(wg2, dtype=np.float32)
    bg2 = np.asarray(bg2, dtype=np.float32)
    w_exp = np.asarray(w_exp, dtype=np.float32)
    b_exp = np.asarray(b_exp, dtype=np.float32)

    # x shipped as [B, 128, FLAT] bf16: rows 0:64 = zero-padded flat
    # image, rows 64:128 = the same shifted +2 elements (the conv's
    # bottom-half K copy) — both SBUF halves land in one full-rate DMA
    xpad = np.zeros((B, C, HP, WP), np.float32)
    xpad[:, :, 1 : H + 1, 1 : W + 1] = x
    flat = xpad.reshape(B, C, FLAT)
    xs = np.zeros((B, 128, FLAT), NPBF16)
    xs[:, 0:64] = flat.astype(NPBF16)
    xs[:, 64:128, 0 : FLAT - 2] = flat[:, :, 2:].astype(NPBF16)

    # wps [128, E, 3(dy), 128]: K top/bottom = taps dx 0/2 on M 0:64 (A),
    # center dx=1 on M 64:128 top (B, bottom zero). Residual identity is
    # folded into every expert's center tap (sum of probs is ~1).
    wt = np.transpose(w_exp, (2, 0, 3, 4, 1))  # [I, E, dy, dx, O]
    wps = np.zeros((128, E, 3, 128), np.float32)
    wps[0:64, :, :, 0:64] = wt[:, :, :, 0, :]
    wps[64:128, :, :, 0:64] = wt[:, :, :, 2, :]
    wps[0:64, :, :, 64:128] = wt[:, :, :, 1, :]
    ii = np.arange(64)
    wps[ii, :, 1, 64 + ii] += 1.0

    gconst = np.zeros((128, 90), np.float32)
    gconst[:, 0:16] = np.concatenate([wg1, wg1], axis=0) / (H * W)
    gconst[0:16, 16] = bg1
    gconst[0:16, 17:25] = wg2
    gconst[16, 17:25] = bg2
    gconst[0:8, 25:89] = b_exp

    shared = {
        "wpsA": np.ascontiguousarray(wps[:, 0:4]).astype(NPBF16),
        "wpsB": np.ascontiguousarray(wps[:, 4:8]).astype(NPBF16),
        "ident": np.eye(128, dtype=NPBF16),
        "gconst": gconst,
    }
    return [
        {"xs": np.ascontiguousarray(xs[SPB * k : SPB * (k + 1)]), **shared}
        for k in range(NCORES)
    ]


def _decode_out(o):
    """[C, H*WP] bf16 -> [C, H, W] f32 (strip the pad columns)."""
    return np.asarray(o, dtype=np.float32).reshape(C, H, WP)[:, :, 0:W]


def kernel(x, wg1, bg1, wg2, bg2, w_exp, b_exp):
    nc = build_program()
    in_maps = host_prep(x, wg1, bg1, wg2, bg2, w_exp, b_exp)
    res = run_bass_kernel_spmd(nc, in_maps, list(range(NCORES)))
    out = np.empty((B, C, H, W), np.float32)
    for k in range(NCORES):
        o = np.asarray(res.results[k]["out"])
        for s in range(SPB):
            out[SPB * k + s] = _decode_out(o[s])
    return out


# revision 31
# speedup vs baseline: 1.0871x; 1.0082x over previous
"""MoE block (top-2 routed 3x3 conv experts) Trainium2 Bass kernel — v6.

Data-parallel over batch, 2 samples per core on 8 cores. The conv is
linear in the kernel, so the top-2 expert kernels are combined with the
routing probabilities first (w_comb = sum_e p_e W_e + I, the +I folding
the residual into the center tap), then one 3x3 SAME conv per sample.
Conv-as-matmul: 6 N=391 matmuls per 6-row pair (dy-major), A-half taps
on psum partitions 0:64, center taps on 64:128 realigned +1 col by the
ACT stage, combined into [64, OBW] out batches.

v6 structure (v4 baseline 118.5us; v5 experiments: TTR crashes HW at
runtime, cross-base SBUF binary ops and 2-bank matmul outs are ISA
errors, gpsimd lacks TensorScalarPtr/TensorReduce):
- Loads x-first on all 3 DMA lanes (SP/gpsimd/ACT), weights slotted
  where they don't delay the GAP->gate->MAC critical path. Per-lane
  effective bandwidth is only ~90-130 GB/s, so s0's chunks are spread
  across all lanes and s1 streams behind them.
- PE warm-up block (~85 throwaway matmuls on wpsA) keeps the HAM clock
  gate at 8/8 through the prologue so real convs start at 2.4 GHz.
- MAC on the PE: wcomb_psum = sum_e (p_e*I)^T @ wps_e — 8 ACT ops build
  p_e*I from a shipped identity (per-partition scale), 8 accumulating
  matmuls, one ACT copy back to bf16. Replaces the serial DVE MAC chain
  (~7-9us/sample) with ~2us ACT + ~1.3us PE.
- b_comb folded into the B-half stage as the ACT activation bias; the
  combine is then a 2-operand tensor_tensor add. DVE does most pairs;
  4 late-s0 pairs go to gpsimd (tensor_tensor IS supported there) via a
  double ACT stage, relieving the DVE in the tight s0 phase.
- GAP: v4-style accumulating windows (ACT tops half / DVE the rest),
  s1's windows and gate emitted between s0 pairs as its chunks land.
"""
import os
import numpy as np
from contextlib import ExitStack

import ml_dtypes

import concourse.bass as bass
import concourse.tile as tile
from concourse import bacc, mybir
from concourse.bass_utils import run_bass_kernel_spmd
import concourse.bass_utils as _bu

if os.environ.get("KLDW") and not getattr(_bu, "_kldw_patched", False):
    _orig_run_command = _bu.run_command

    def _run_command(cmd, **kw):
        cmd = [
            c.replace("--enable-ldw-opt=false", "--enable-ldw-opt=true")
            if isinstance(c, str) else c
            for c in cmd
        ]
        return _orig_run_command(cmd, **kw)

    _bu.run_command = _run_command
    _bu._kldw_patched = True

F32 = mybir.dt.float32
BF16 = mybir.dt.bfloat16
AX = mybir.AxisListType
OP = mybir.AluOpType
ACTF = mybir.ActivationFunctionType

B, C, H, W, E, GH = 16, 64, 128, 128, 8, 16
NCORES = 8
SPB = B // NCORES          # samples per core
HP, WP = H + 2, W + 2      # 130
FLAT = HP * WP             # 16900
QC = FLAT // 4             # x-load chunk size (4225 flat elements)
OBW = 24 * WP              # out batch region width (3120)
NPAIR = 22                 # 21 six-row pairs + one trailing 2-row tile
WARM_MM = 85               # PE warm-up matmuls (span the prologue)
GPS_PAIRS_S0 = {10, 13, 16, 19}   # s0 pairs whose combine runs on gpsimd

NPBF16 = ml_dtypes.bfloat16

_cache = {}

# GAP windows over the flat layout (pad zeros included): top copy
# (partitions 0:64) covers flat[0:2QC+2), bottom copy (64:128, shifted
# +2) covers flat[2QC+2:FLAT). Each op accumulates into a part slot;
# the gate matmul's stacked wg1x2 sums the two partition halves.
GAP_TOP = [(0, QC, 0), (QC, 2 * QC + 2, 1)]
GAP_BOT = [(2 * QC, 3 * QC, 0), (3 * QC, FLAT, 1)]
QH = QC // 2
GAP_BOT4 = [
    (2 * QC, 2 * QC + QH, 0),
    (2 * QC + QH, 3 * QC, 1),
    (3 * QC, 3 * QC + QH, 2),
    (3 * QC + QH, FLAT, 3),
]


def _emit_gap_op(nc, pools, XX, part, win, is_bot, eng):
    a, b, slot = win
    lo, hi = (64, 128) if is_bot else (0, 64)
    src = XX[lo:hi, a:b]
    dst = pools["scrS" if eng == "act" else "scrD"][lo:hi]
    acc = part[lo:hi, slot : slot + 1]
    if eng == "act":
        return nc.scalar.activation(
            dst[:, 0 : b - a], src, ACTF.Copy, accum_out=acc
        )
    return nc.vector.tensor_scalar(
        dst[:, 0 : b - a], src, 0.0, 0.0, OP.add, OP.add, accum_out=acc
    )


def _emit_gate(nc, pools, s, pooled, consts, h_ext):
    """Gate MLP + softmax + top-2 for one sample (all f32).

    exp-without-max-sub (logits are small); folds the top-2 mask and
    renormalization: w8 = (u>=m2)*u / (sum((u>=m2)*u) + sum(u)*1e-8).
    Returns (wb_sb [128,E] f32 per-partition probs, b_comb [C,1]).
    """
    f = pools
    g = f["gate"]
    wg1x2_sb, bg1_sb, wg2_sb, bexp_sb, ones = consts
    n = lambda base: f"{base}{s}"

    h_ps = f["gpsum"].tile([GH, 1], F32, tag="cps", name=n("h_ps"))
    nc.tensor.matmul(h_ps[:], lhsT=wg1x2_sb, rhs=pooled[:], start=True, stop=True)
    nc.vector.tensor_scalar(h_ext[0:GH, :], h_ps[:], bg1_sb, 0.0, OP.add, OP.max)

    lg_ps = f["gpsum"].tile([1, E], F32, tag="cps", name=n("lg_ps"))
    nc.tensor.matmul(lg_ps[:], lhsT=h_ext[:], rhs=wg2_sb, start=True, stop=True)

    u = g.tile([1, E], F32, tag="u", name=n("u"))
    nc.scalar.activation(u[:], lg_ps[:], ACTF.Exp)
    usum = g.tile([1, 1], F32, tag="usum", name=n("usum"))
    nc.vector.tensor_reduce(usum[:], u[:], axis=AX.X, op=OP.add)
    m1p = g.tile([1, 1], F32, tag="m1p", name=n("m1p"))
    nc.vector.tensor_reduce(m1p[:], u[:], axis=AX.X, op=OP.max)
    pm = g.tile([1, E], F32, tag="pm", name=n("pm"))
    nc.vector.scalar_tensor_tensor(pm[:], u[:], m1p[:], u[:], op0=OP.is_lt, op1=OP.mult)
    m2 = g.tile([1, 1], F32, tag="m2", name=n("m2"))
    nc.vector.tensor_reduce(m2[:], pm[:], axis=AX.X, op=OP.max)
    spv = g.tile([1, E], F32, tag="spv", name=n("spv"))
    nc.vector.scalar_tensor_tensor(spv[:], u[:], m2[:], u[:], op0=OP.is_ge, op1=OP.mult)
    dsum = g.tile([1, 1], F32, tag="dsum", name=n("dsum"))
    nc.vector.tensor_reduce(dsum[:], spv[:], axis=AX.X, op=OP.add)
    dd = g.tile([1, 1], F32, tag="dd", name=n("dd"))
    nc.vector.scalar_tensor_tensor(dd[:], usum[:], 1e-8, dsum[:], op0=OP.mult, op1=OP.add)
    rr = g.tile([1, 1], F32, tag="rr", name=n("rr"))
    nc.vector.reciprocal(rr[:], dd[:])
    w8 = g.tile([1, E], F32, tag="w8", name=n("w8"))
    nc.vector.tensor_scalar_mul(w8[:], spv[:], rr[:])

    # broadcast w8 down all 128 partitions, then stage to SBUF for MACs
    wb_ps = f["gpsum"].tile([128, E], F32, tag="cps", name=n("wb_ps"))
    nc.tensor.matmul(wb_ps[:], lhsT=ones[:], rhs=w8[:], start=True, stop=True)
    wb_sb = g.tile([128, E], F32, tag="wb_sb", name=n("wb_sb"))
    nc.vector.tensor_copy(wb_sb[:], wb_ps[:])

    # combined bias: b_comb = b_exp^T @ w8^T
    w8c_ps = f["gpsum"].tile([E, 1], F32, tag="cps", name=n("w8c_ps"))
    nc.tensor.matmul(w8c_ps[:], lhsT=w8[:], rhs=ones[:, 0:1], start=True, stop=True)
    w8col = g.tile([E, 1], F32, tag="w8col", name=n("w8col"))
    nc.vector.tensor_copy(w8col[:], w8c_ps[:])
    bc_ps = f["gpsum"].tile([C, 1], F32, tag="cps", name=n("bc_ps"))
    nc.tensor.matmul(bc_ps[:], lhsT=bexp_sb, rhs=w8col[:], start=True, stop=True)
    b_comb = g.tile([C, 1], F32, tag="b_comb", name=n("b_comb"))
    nc.vector.tensor_copy(b_comb[:], bc_ps[:])
    return wb_sb, b_comb


def _emit_mac_pe(nc, pools, s, wb_sb, wpsA_sb, wpsB_sb, ident_sb):
    """wcomb = sum_e p_e wps_e on the PE: 8 accumulating matmuls with
    lhsT = p_e*I (built by ACT from the shipped identity with the
    per-partition probability as activation scale). Residual identity is
    pre-folded into every expert's center-tap B-half on the host."""
    f = pools
    pI = f["wcomb"].tile([128, E, 128], BF16, tag="pI", name=f"pI{s}")
    for e in range(E):
        nc.scalar.activation(
            pI[:, e, :], ident_sb[:], ACTF.Copy, scale=wb_sb[:, e : e + 1]
        )
    wcps = f["gpsum"].tile([128, 384], F32, tag="cps", name=f"wcps{s}")
    for e in range(E):
        src = wpsA_sb[:, e] if e < 4 else wpsB_sb[:, e - 4]
        nc.tensor.matmul(
            wcps[:],
            lhsT=pI[:, e, :],
            rhs=src.rearrange("p a b -> p (a b)"),
            start=(e == 0),
            stop=(e == E - 1),
        )
    wcombr = f["wcomb"].tile([128, 3, 128], BF16, tag="wcombr", name=f"wcombr{s}")
    nc.scalar.activation(
        wcombr[:].rearrange("p a b -> p (a b)"), wcps[:], ACTF.Copy
    )
    return wcombr


def _emit_pair(nc, pools, s, p, XX, wcombr, b_comb, ob, ocol, gps):
    """Conv for pair p: 6 matmuls (dy-major, N=ncol+1 so the stage's +1
    col realignment only reads written psum) into a 2-bank PSUM tile.
    ACT stages the B half with b_comb as activation bias; the combine is
    then obv = psA + stB (DVE tensor_tensor, or gpsimd via an extra ACT
    stage of the A half — gpsimd has no PSUM access)."""
    f = pools
    r0 = 6 * p
    last = p == NPAIR - 1
    nt = 1 if last else 2      # psum banks (3-row tiles) in this pair
    nr = 2 if last else 6      # rows
    ps = f["cpsum"].tile([128, 2, 512], F32, tag="cps", name=f"cps{s}_{p}")
    ncol = (nr // nt) * WP
    trows = nr // nt
    for dyi in range(3):
        for t in range(nt):
            ra = r0 + t * trows + dyi
            nc.tensor.matmul(
                ps[:, t, 0 : ncol + 1],
                lhsT=wcombr[:, dyi, :],
                rhs=XX[:, ra * WP : ra * WP + ncol + 1],
                start=(dyi == 0),
                stop=(dyi == 2),
            )
    obv = ob[:, ocol : ocol + nt * ncol].rearrange("p (t c) -> p t c", c=ncol)
    stB = f["stage"].tile([64, 2, 390], BF16, tag="stB", name=f"stB{s}_{p}")
    nc.scalar.activation(stB[:, 0:nt, 0:ncol], ps[64:128, 0:nt, 1 : ncol + 1], ACTF.Copy)
    if gps is not None:
        # gpsimd combine (TensorTensor only there, no PSUM access): ACT
        # stages the A half too; bias comes from the per-sample broadcast
        # tile in a second add
        stA = f["stage"].tile([64, 2, 390], BF16, tag="stA", name=f"stA{s}_{p}")
        nc.scalar.activation(stA[:, 0:nt, 0:ncol], ps[0:64, 0:nt, 0:ncol], ACTF.Copy)
        nc.gpsimd.tensor_tensor(
            obv, stA[:, 0:nt, 0:ncol], stB[:, 0:nt, 0:ncol], op=OP.add
        )
        return nc.gpsimd.tensor_tensor(obv, obv, gps[:, 0:nt, 0:ncol], op=OP.add)
    return nc.vector.scalar_tensor_tensor(
        obv,
        ps[0:64, 0:nt, 0:ncol],
        b_comb[:],
        stB[:, 0:nt, 0:ncol],
        op0=OP.add,
        op1=OP.add,
    )


def build_program():
    if "nc" in _cache:
        return _cache["nc"]
    nc = bacc.Bacc("TRN2", target_bir_lowering=False, debug=False, enable_asserts=False)
    xs_ap = nc.dram_tensor("xs", [SPB, 128, FLAT], BF16, kind="ExternalInput").ap()
    wpsA_d = nc.dram_tensor("wpsA", [128, E // 2, 3, 128], BF16, kind="ExternalInput").ap()
    wpsB_d = nc.dram_tensor("wpsB", [128, E // 2, 3, 128], BF16, kind="ExternalInput").ap()
    ident_d = nc.dram_tensor("ident", [128, 128], BF16, kind="ExternalInput").ap()
    ident_d = nc.dram_tensor("ident", [128, 128], BF16, kind="ExternalInput").ap()
    gconst_d = nc.dram_tensor("gconst", [128, 90], F32, kind="ExternalInput").ap()
    out_ap = nc.dram_tensor("out", [SPB, C, H * WP], BF16, kind="ExternalOutput").ap()

    with tile.TileContext(nc) as tc, ExitStack() as ctx:
        pools = {
            "const": ctx.enter_context(tc.tile_pool(name="const", bufs=1)),
            "xx": ctx.enter_context(tc.tile_pool(name="xx", bufs=SPB)),
            "gate": ctx.enter_context(tc.tile_pool(name="gate", bufs=3)),
            "wcomb": ctx.enter_context(tc.tile_pool(name="wcomb", bufs=3)),
            "stage": ctx.enter_context(tc.tile_pool(name="stage", bufs=6)),
            "cpsum": ctx.enter_context(tc.tile_pool(name="cpsum", bufs=3, space="PSUM")),
            "gpsum": ctx.enter_context(tc.tile_pool(name="gpsum", bufs=2, space="PSUM")),
        }
        cp = pools["const"]
        # +4 zeroed pad cols so the tail tile's widened matmul read stays
        # in bounds
        XX0 = pools["xx"].tile([128, FLAT + 4], BF16, tag="XX", name="XX0")
        XX1 = pools["xx"].tile([128, FLAT + 4], BF16, tag="XX", name="XX1")
        nc.vector.memset(XX0[:, FLAT : FLAT + 4], 0.0)
        nc.vector.memset(XX1[:, FLAT : FLAT + 4], 0.0)
        gconst_sb = cp.tile([128, 90], F32)
        ones = cp.tile([1, 128], F32)
        nc.gpsimd.memset(ones[:], 1.0)
        wpsA_sb = cp.tile([128, E // 2, 3, 128], BF16)
        wpsB_sb = cp.tile([128, E // 2, 3, 128], BF16)
        ident_sb = cp.tile([128, 128], BF16)
        ident_sb = cp.tile([128, 128], BF16)
        pools["scrD"] = cp.tile([128, QC + 2], BF16, name="scrD")
        pools["scrS"] = cp.tile([128, QC + 2], BF16, name="scrS")

        # ---- loads + prologue compute, interleaved so each consumer's
        # queue drain covers only the transfers it actually needs (a
        # consumer emitted after later triggers on a lane waits for ALL
        # of them - this drain effect, not bandwidth, dominated the v4/v6
        # prologues) ----
        C3A = 3 * QC + 2113
        nc.scalar.dma_start(wpsA_sb[:], wpsA_d[:])       # warmup needs it
        nc.scalar.dma_start(ident_sb[:], ident_d[:])

        # PE warm-up: no-DMA scratch matmuls start ~2us (HAM to 8/8),
        # then chunk-gated batches self-time the warm window to the load
        warm_sc = cp.tile([128, 384], BF16, name="warm_sc")
        nc.gpsimd.memset(warm_sc[:], 0.25)
        warm_ps = pools["gpsum"].tile([128, 384], F32, tag="cps", name="warm_ps")

        def emit_warm(n, rhs):
            for _ in range(n):
                nc.tensor.matmul(
                    warm_ps[:], lhsT=warm_sc[:, 0:128], rhs=rhs,
                    start=True, stop=True,
                )

        emit_warm(30, warm_sc[:])

        part0 = pools["gate"].tile([128, 2], F32, tag="part", name="part0")
        h_ext0 = pools["gate"].tile([GH + 1, 1], F32, tag="h_ext", name="h_ext0")
        h_ext1 = pools["gate"].tile([GH + 1, 1], F32, tag="h_ext", name="h_ext1")

        nc.sync.dma_start(XX0[:, 0:QC], xs_ap[0, :, 0:QC])
        nc.sync.dma_start(h_ext0[GH : GH + 1, 0:1], ones[0:1, 0:1])
        nc.sync.dma_start(h_ext1[GH : GH + 1, 0:1], ones[0:1, 0:1])
        w1i = _emit_gap_op(nc, pools, XX0, part0, GAP_TOP[0], is_bot=False, eng="act")
        emit_warm(8, XX0[:, 0:384])

        nc.gpsimd.dma_start(gconst_sb[:], gconst_d[:])
        nc.gpsimd.dma_start(XX0[:, QC : 2 * QC], xs_ap[0, :, QC : 2 * QC])
        nc.scalar.dma_start(XX0[:, 2 * QC : 3 * QC], xs_ap[0, :, 2 * QC : 3 * QC])
        w2i = _emit_gap_op(nc, pools, XX0, part0, GAP_TOP[1], is_bot=False, eng="dve")
        w3i = _emit_gap_op(nc, pools, XX0, part0, GAP_BOT[0], is_bot=True, eng="dve")
        emit_warm(8, XX0[:, QC : QC + 384])
        emit_warm(8, XX0[:, 2 * QC : 2 * QC + 384])

        nc.gpsimd.dma_start(XX0[:, C3A:FLAT], xs_ap[0, :, C3A:FLAT])
        nc.sync.dma_start(XX0[:, 3 * QC : C3A], xs_ap[0, :, 3 * QC : C3A])
        w4i = _emit_gap_op(nc, pools, XX0, part0, GAP_BOT[1], is_bot=True, eng="act")
        emit_warm(8, XX0[:, 3 * QC : 3 * QC + 384])

        nc.scalar.dma_start(wpsB_sb[:], wpsB_d[:])

        wg1x2_sb = gconst_sb[:, 0:16]
        bg1_sb = gconst_sb[0:16, 16:17]
        wg2_sb = gconst_sb[0:17, 17:25]
        bexp_sb = gconst_sb[0:8, 25:89]
        consts = (wg1x2_sb, bg1_sb, wg2_sb, bexp_sb, ones)

        pooled0 = pools["gate"].tile([128, 1], F32, tag="pooled", name="pooled0")
        nc.vector.tensor_reduce(pooled0, part0[:], axis=AX.X, op=OP.add)
        wb0, bcomb0 = _emit_gate(nc, pools, 0, pooled0, consts, h_ext0)
        wcombr0 = _emit_mac_pe(nc, pools, 0, wb0, wpsA_sb, wpsB_sb, ident_sb)
        zb = cp.tile([64, 2, 390], BF16, name="zb")
        nc.gpsimd.memset(zb[:], 0.0)
        bB0 = pools["gate"].tile([64, 2, 390], BF16, tag="bB", name="bB0")
        nc.vector.scalar_tensor_tensor(
            bB0[:], zb[:], bcomb0[:], zb[:], op0=OP.add, op1=OP.add
        )

        # ---- s1 x loads: each trigger HARD-pinned after an s0 GAP op so
        # no s0 consumer's queue drain can end up covering s1 transfers
        # (the scheduler otherwise floats these triggers early) ----
        def pin_trig(ti, gi):
            tile.add_dep_helper(
                ti.ins, gi.ins, sync=True,
                reason="s1 load held until s0 GAP consumed its lane",
            )

        pin_trig(nc.gpsimd.dma_start(XX1[:, 0:QC], xs_ap[1, :, 0:QC]), w1i)
        pin_trig(nc.sync.dma_start(XX1[:, QC : 2 * QC], xs_ap[1, :, QC : 2 * QC]), w2i)
        pin_trig(
            nc.scalar.dma_start(XX1[:, 2 * QC : 3 * QC], xs_ap[1, :, 2 * QC : 3 * QC]),
            w3i,
        )
        pin_trig(nc.sync.dma_start(XX1[:, 3 * QC : C3A], xs_ap[1, :, 3 * QC : C3A]), w4i)
        pin_trig(nc.gpsimd.dma_start(XX1[:, C3A:FLAT], xs_ap[1, :, C3A:FLAT]), w4i)

        part1 = pools["gate"].tile([128, 4], F32, tag="part", name="part1")
        nc.gpsimd.memset(part1[0:64, 2:4], 0.0)
        s1_state = {}

        def s1_hook(p, comb):
            def pin(gi):
                tile.add_dep_helper(
                    gi.ins, comb.ins, sync=False,
                    reason="s1 prep slotted after this pair's combine",
                )
            if p == 4:
                pin(_emit_gap_op(nc, pools, XX1, part1, GAP_TOP[0], is_bot=False, eng="act"))
            elif p == 7:
                pin(_emit_gap_op(nc, pools, XX1, part1, GAP_TOP[1], is_bot=False, eng="dve"))
            elif p in (9, 11, 13, 14):
                k = {9: 0, 11: 1, 13: 2, 14: 3}[p]
                pin(_emit_gap_op(nc, pools, XX1, part1, GAP_BOT4[k], is_bot=True, eng="dve"))
            elif p == 15:
                pooled1 = pools["gate"].tile(
                    [128, 1], F32, tag="pooled", name="pooled1"
                )
                pin(nc.vector.tensor_reduce(pooled1, part1[:], axis=AX.X, op=OP.add))
                wb1, bcomb1 = _emit_gate(nc, pools, 1, pooled1, consts, h_ext1)
                s1_state["bcomb"] = bcomb1
                s1_state["wcombr"] = _emit_mac_pe(
                    nc, pools, 1, wb1, wpsA_sb, wpsB_sb, ident_sb
                )
                bB1 = pools["gate"].tile([64, 2, 390], BF16, tag="bB", name="bB1")
                nc.vector.scalar_tensor_tensor(
                    bB1[:], zb[:], bcomb1[:], zb[:], op0=OP.add, op1=OP.add
                )
                s1_state["bB"] = bB1

        # out batching: one [64, OBW] buffer per 24-row batch (batch 5 is
        # 8 rows); s0 batches drain on SP, s1 batches on gpsimd
        obstate = {0: [None, 0], 1: [None, 0]}

        bBmap = {}

        def emit_sample_pairs(s, XX, wcombr, bcomb, rng, hook=None):
            for p in rng:
                batch = min(p // 4, 5)
                ob, ocol = obstate[s]
                if ob is None:
                    ob = pools["stage"].tile(
                        [64, OBW], BF16, tag="ob", name=f"ob{s}_{batch}", bufs=3
                    )
                    obstate[s] = [ob, 0]
                    ocol = 0
                gps = bBmap.get(s) if (s == 0 and p in GPS_PAIRS_S0) else None
                comb = _emit_pair(nc, pools, s, p, XX, wcombr, bcomb, ob, ocol, gps)
                ocol += 780 if p < NPAIR - 1 else 260
                obstate[s][1] = ocol
                bcols = OBW if batch < 5 else 1040
                if ocol == bcols:
                    lane = nc.sync if s == 0 else nc.gpsimd
                    lane.dma_start(
                        out_ap[s, :, 24 * batch * WP : 24 * batch * WP + bcols],
                        ob[:, 0:bcols],
                    )
                    obstate[s] = [None, 0]
                if hook is not None:
                    hook(p, comb)

        bBmap[0] = bB0
        emit_sample_pairs(0, XX0, wcombr0, bcomb0, range(NPAIR), s1_hook)
        emit_sample_pairs(
            1, XX1, s1_state["wcombr"], s1_state["bcomb"], range(NPAIR)
        )

    nc.compile()
    _cache["nc"] = nc
    return nc


def host_prep(x, wg1, bg1, wg2, bg2, w_exp, b_exp):
    """Host-side layout prep + per-core sharding. Returns in_maps list."""
    x = np.asarray(x, dtype=np.float32)
    wg1 = np.asarray(wg1, dtype=np.float32)
    bg1 = np.asarray(bg1, dtype=np.float32)
    wg2 = np.asarray(wg2, dtype=np.float32)
    bg2 = np.asarray(bg2, dtype=np.float32)
    w_exp = np.asarray(w_exp, dtype=np.float32)
    b_exp = np.asarray(b_exp, dtype=np.float32)

    # x shipped as [B, 128, FLAT] bf16: rows 0:64 = zero-padded flat
    # image, rows 64:128 = the same shifted +2 elements (the conv's
    # bottom-half K copy) — both SBUF halves land in one full-rate DMA
    xpad = np.zeros((B, C, HP, WP), np.float32)
    xpad[:, :, 1 : H + 1, 1 : W + 1] = x
    flat = xpad.reshape(B, C, FLAT)
    xs = np.zeros((B, 128, FLAT), NPBF16)
    xs[:, 0:64] = flat.astype(NPBF16)
    xs[:, 64:128, 0 : FLAT - 2] = flat[:, :, 2:].astype(NPBF16)

    # wps [128, E, 3(dy), 128]: K top/bottom = taps dx 0/2 on M 0:64 (A),
    # center dx=1 on M 64:128 top (B, bottom zero). Residual identity is
    # folded into every expert's center tap (sum of probs is ~1).
    wt = np.transpose(w_exp, (2, 0, 3, 4, 1))  # [I, E, dy, dx, O]
    wps = np.zeros((128, E, 3, 128), np.float32)
    wps[0:64, :, :, 0:64] = wt[:, :, :, 0, :]
    wps[64:128, :, :, 0:64] = wt[:, :, :, 2, :]
    wps[0:64, :, :, 64:128] = wt[:, :, :, 1, :]
    ii = np.arange(64)
    wps[ii, :, 1, 64 + ii] += 1.0

    gconst = np.zeros((128, 90), np.float32)
    gconst[:, 0:16] = np.concatenate([wg1, wg1], axis=0) / (H * W)
    gconst[0:16, 16] = bg1
    gconst[0:16, 17:25] = wg2
    gconst[16, 17:25] = bg2
    gconst[0:8, 25:89] = b_exp

    shared = {
        "wpsA": np.ascontiguousarray(wps[:, 0:4]).astype(NPBF16),
        "wpsB": np.ascontiguousarray(wps[:, 4:8]).astype(NPBF16),
        "ident": np.eye(128, dtype=NPBF16),
        "gconst": gconst,
    }
    return [
        {"xs": np.ascontiguousarray(xs[SPB * k : SPB * (k + 1)]), **shared}
        for k in range(NCORES)
    ]


def _decode_out(o):
    """[C, H*WP] bf16 -> [C, H, W] f32 (strip the pad columns)."""
    return np.asarray(o, dtype=np.float32).reshape(C, H, WP)[:, :, 0:W]


def kernel(x, wg1, bg1, wg2, bg2, w_exp, b_exp):
    nc = build_program()
    in_maps = host_prep(x, wg1, bg1, wg2, bg2, w_exp, b_exp)
    res = run_bass_kernel_spmd(nc, in_maps, list(range(NCORES)))
    out = np.empty((B, C, H, W), np.float32)
    for k in range(NCORES):
        o = np.asarray(res.results[k]["out"])
        for s in range(SPB):
            out[SPB * k + s] = _decode_out(o[s])
    return out


# revision 33
# speedup vs baseline: 1.0950x; 1.0073x over previous
"""MoE block (top-2 routed 3x3 conv experts) Trainium2 Bass kernel — v6.

Data-parallel over batch, 2 samples per core on 8 cores. The conv is
linear in the kernel, so the top-2 expert kernels are combined with the
routing probabilities first (w_comb = sum_e p_e W_e + I, the +I folding
the residual into the center tap), then one 3x3 SAME conv per sample.
Conv-as-matmul: 6 N=391 matmuls per 6-row pair (dy-major), A-half taps
on psum partitions 0:64, center taps on 64:128 realigned +1 col by the
ACT stage, combined into [64, OBW] out batches.

v6 structure (v4 baseline 118.5us; v5 experiments: TTR crashes HW at
runtime, cross-base SBUF binary ops and 2-bank matmul outs are ISA
errors, gpsimd lacks TensorScalarPtr/TensorReduce):
- Loads x-first on all 3 DMA lanes (SP/gpsimd/ACT), weights slotted
  where they don't delay the GAP->gate->MAC critical path. Per-lane
  effective bandwidth is only ~90-130 GB/s, so s0's chunks are spread
  across all lanes and s1 streams behind them.
- PE warm-up block (~85 throwaway matmuls on wpsA) keeps the HAM clock
  gate at 8/8 through the prologue so real convs start at 2.4 GHz.
- MAC on the PE: wcomb_psum = sum_e (p_e*I)^T @ wps_e — 8 ACT ops build
  p_e*I from a shipped identity (per-partition scale), 8 accumulating
  matmuls, one ACT copy back to bf16. Replaces the serial DVE MAC chain
  (~7-9us/sample) with ~2us ACT + ~1.3us PE.
- b_comb folded into the B-half stage as the ACT activation bias; the
  combine is then a 2-operand tensor_tensor add. DVE does most pairs;
  4 late-s0 pairs go to gpsimd (tensor_tensor IS supported there) via a
  double ACT stage, relieving the DVE in the tight s0 phase.
- GAP: v4-style accumulating windows (ACT tops half / DVE the rest),
  s1's windows and gate emitted between s0 pairs as its chunks land.
"""
import os
import numpy as np
from contextlib import ExitStack

import ml_dtypes

import concourse.bass as bass
import concourse.tile as tile
from concourse import bacc, mybir
from concourse.bass_utils import run_bass_kernel_spmd
import concourse.bass_utils as _bu

if os.environ.get("KLDW") and not getattr(_bu, "_kldw_patched", False):
    _orig_run_command = _bu.run_command

    def _run_command(cmd, **kw):
        cmd = [
            c.replace("--enable-ldw-opt=false", "--enable-ldw-opt=true")
            if isinstance(c, str) else c
            for c in cmd
        ]
        return _orig_run_command(cmd, **kw)

    _bu.run_command = _run_command
    _bu._kldw_patched = True

F32 = mybir.dt.float32
BF16 = mybir.dt.bfloat16
AX = mybir.AxisListType
OP = mybir.AluOpType
ACTF = mybir.ActivationFunctionType

B, C, H, W, E, GH = 16, 64, 128, 128, 8, 16
NCORES = 8
SPB = B // NCORES          # samples per core
HP, WP = H + 2, W + 2      # 130
FLAT = HP * WP             # 16900
QC = FLAT // 4             # x-load chunk size (4225 flat elements)
OBW = 24 * WP              # out batch region width (3120)
NPAIR = 22                 # 21 six-row pairs + one trailing 2-row tile
WARM_MM = 85               # PE warm-up matmuls (span the prologue)
GPS_PAIRS_S0 = {10, 13, 16, 19}   # s0 pairs whose combine runs on gpsimd

NPBF16 = ml_dtypes.bfloat16

_cache = {}

# GAP windows over the flat layout (pad zeros included): top copy
# (partitions 0:64) covers flat[0:2QC+2), bottom copy (64:128, shifted
# +2) covers flat[2QC+2:FLAT). Each op accumulates into a part slot;
# the gate matmul's stacked wg1x2 sums the two partition halves.
GAP_TOP = [(0, QC, 0), (QC, 2 * QC + 2, 1)]
GAP_BOT = [(2 * QC, 3 * QC, 0), (3 * QC, FLAT, 1)]
QH = QC // 2
GAP_BOT4 = [
    (2 * QC, 2 * QC + QH, 0),
    (2 * QC + QH, 3 * QC, 1),
    (3 * QC, 3 * QC + QH, 2),
    (3 * QC + QH, FLAT, 3),
]


def _emit_gap_op(nc, pools, XX, part, win, is_bot, eng):
    a, b, slot = win
    lo, hi = (64, 128) if is_bot else (0, 64)
    src = XX[lo:hi, a:b]
    dst = pools["scrS" if eng == "act" else "scrD"][lo:hi]
    acc = part[lo:hi, slot : slot + 1]
    if eng == "act":
        return nc.scalar.activation(
            dst[:, 0 : b - a], src, ACTF.Copy, accum_out=acc
        )
    return nc.vector.tensor_scalar(
        dst[:, 0 : b - a], src, 0.0, 0.0, OP.add, OP.add, accum_out=acc
    )


def _emit_gate(nc, pools, s, pooled, consts, h_ext):
    """Gate MLP + softmax + top-2 for one sample (all f32).

    exp-without-max-sub (logits are small); folds the top-2 mask and
    renormalization: w8 = (u>=m2)*u / (sum((u>=m2)*u) + sum(u)*1e-8).
    Returns (wb_sb [128,E] f32 per-partition probs, b_comb [C,1]).
    """
    f = pools
    g = f["gate"]
    wg1x2_sb, bg1_sb, wg2_sb, bexp_sb, ones = consts
    n = lambda base: f"{base}{s}"

    h_ps = f["gpsum"].tile([GH, 1], F32, tag="cps", name=n("h_ps"))
    nc.tensor.matmul(h_ps[:], lhsT=wg1x2_sb, rhs=pooled[:], start=True, stop=True)
    nc.vector.tensor_scalar(h_ext[0:GH, :], h_ps[:], bg1_sb, 0.0, OP.add, OP.max)

    lg_ps = f["gpsum"].tile([1, E], F32, tag="cps", name=n("lg_ps"))
    nc.tensor.matmul(lg_ps[:], lhsT=h_ext[:], rhs=wg2_sb, start=True, stop=True)

    u = g.tile([1, E], F32, tag="u", name=n("u"))
    nc.scalar.activation(u[:], lg_ps[:], ACTF.Exp)
    usum = g.tile([1, 1], F32, tag="usum", name=n("usum"))
    nc.vector.tensor_reduce(usum[:], u[:], axis=AX.X, op=OP.add)
    m1p = g.tile([1, 1], F32, tag="m1p", name=n("m1p"))
    nc.vector.tensor_reduce(m1p[:], u[:], axis=AX.X, op=OP.max)
    pm = g.tile([1, E], F32, tag="pm", name=n("pm"))
    nc.vector.scalar_tensor_tensor(pm[:], u[:], m1p[:], u[:], op0=OP.is_lt, op1=OP.mult)
    m2 = g.tile([1, 1], F32, tag="m2", name=n("m2"))
    nc.vector.tensor_reduce(m2[:], pm[:], axis=AX.X, op=OP.max)
    spv = g.tile([1, E], F32, tag="spv", name=n("spv"))
    nc.vector.scalar_tensor_tensor(spv[:], u[:], m2[:], u[:], op0=OP.is_ge, op1=OP.mult)
    dsum = g.tile([1, 1], F32, tag="dsum", name=n("dsum"))
    nc.vector.tensor_reduce(dsum[:], spv[:], axis=AX.X, op=OP.add)
    dd = g.tile([1, 1], F32, tag="dd", name=n("dd"))
    nc.vector.scalar_tensor_tensor(dd[:], usum[:], 1e-8, dsum[:], op0=OP.mult, op1=OP.add)
    rr = g.tile([1, 1], F32, tag="rr", name=n("rr"))
    nc.vector.reciprocal(rr[:], dd[:])
    w8 = g.tile([1, E], F32, tag="w8", name=n("w8"))
    nc.vector.tensor_scalar_mul(w8[:], spv[:], rr[:])

    # broadcast w8 down all 128 partitions, then stage to SBUF for MACs
    wb_ps = f["gpsum"].tile([128, E], F32, tag="cps", name=n("wb_ps"))
    nc.tensor.matmul(wb_ps[:], lhsT=ones[:], rhs=w8[:], start=True, stop=True)
    wb_sb = g.tile([128, E], F32, tag="wb_sb", name=n("wb_sb"))
    nc.vector.tensor_copy(wb_sb[:], wb_ps[:])

    # combined bias: b_comb = b_exp^T @ w8^T
    w8c_ps = f["gpsum"].tile([E, 1], F32, tag="cps", name=n("w8c_ps"))
    nc.tensor.matmul(w8c_ps[:], lhsT=w8[:], rhs=ones[:, 0:1], start=True, stop=True)
    w8col = g.tile([E, 1], F32, tag="w8col", name=n("w8col"))
    nc.vector.tensor_copy(w8col[:], w8c_ps[:])
    bc_ps = f["gpsum"].tile([C, 1], F32, tag="cps", name=n("bc_ps"))
    nc.tensor.matmul(bc_ps[:], lhsT=bexp_sb, rhs=w8col[:], start=True, stop=True)
    b_comb = g.tile([C, 1], F32, tag="b_comb", name=n("b_comb"))
    nc.vector.tensor_copy(b_comb[:], bc_ps[:])
    return wb_sb, b_comb


def _emit_mac_pe(nc, pools, s, wb_sb, wpsA_sb, wpsB_sb, ident_sb):
    """wcomb = sum_e p_e wps_e on the PE: 8 accumulating matmuls with
    lhsT = p_e*I (built by ACT from the shipped identity with the
    per-partition probability as activation scale). Residual identity is
    pre-folded into every expert's center-tap B-half on the host."""
    f = pools
    pI = f["wcomb"].tile([128, E, 128], BF16, tag="pI", name=f"pI{s}")
    for e in range(E):
        nc.scalar.activation(
            pI[:, e, :], ident_sb[:], ACTF.Copy, scale=wb_sb[:, e : e + 1]
        )
    wcps = f["gpsum"].tile([128, 384], F32, tag="cps", name=f"wcps{s}")
    for e in range(E):
        src = wpsA_sb[:, e] if e < 4 else wpsB_sb[:, e - 4]
        nc.tensor.matmul(
            wcps[:],
            lhsT=pI[:, e, :],
            rhs=src.rearrange("p a b -> p (a b)"),
            start=(e == 0),
            stop=(e == E - 1),
        )
    wcombr = f["wcomb"].tile([128, 3, 128], BF16, tag="wcombr", name=f"wcombr{s}")
    nc.scalar.activation(
        wcombr[:].rearrange("p a b -> p (a b)"), wcps[:], ACTF.Copy
    )
    return wcombr


def _emit_pair(nc, pools, s, p, XX, wcombr, b_comb, ob, ocol, gps):
    """Conv for pair p: 6 matmuls (dy-major, N=ncol+1 so the stage's +1
    col realignment only reads written psum) into a 2-bank PSUM tile.
    ACT stages the B half with b_comb as activation bias; the combine is
    then obv = psA + stB (DVE tensor_tensor, or gpsimd via an extra ACT
    stage of the A half — gpsimd has no PSUM access)."""
    f = pools
    r0 = 6 * p
    last = p == NPAIR - 1
    nt = 1 if last else 2      # psum banks (3-row tiles) in this pair
    nr = 2 if last else 6      # rows
    ps = f["cpsum"].tile([128, 2, 512], F32, tag="cps", name=f"cps{s}_{p}")
    ncol = (nr // nt) * WP
    trows = nr // nt
    for dyi in range(3):
        for t in range(nt):
            ra = r0 + t * trows + dyi
            nc.tensor.matmul(
                ps[:, t, 0 : ncol + 1],
                lhsT=wcombr[:, dyi, :],
                rhs=XX[:, ra * WP : ra * WP + ncol + 1],
                start=(dyi == 0),
                stop=(dyi == 2),
            )
    obv = ob[:, ocol : ocol + nt * ncol].rearrange("p (t c) -> p t c", c=ncol)
    stB = f["stage"].tile([64, 2, 390], BF16, tag="stB", name=f"stB{s}_{p}")
    nc.scalar.activation(stB[:, 0:nt, 0:ncol], ps[64:128, 0:nt, 1 : ncol + 1], ACTF.Copy)
    if gps is not None:
        # gpsimd combine (TensorTensor only there, no PSUM access): ACT
        # stages the A half too; bias comes from the per-sample broadcast
        # tile in a second add
        stA = f["stage"].tile([64, 2, 390], BF16, tag="stA", name=f"stA{s}_{p}")
        nc.scalar.activation(stA[:, 0:nt, 0:ncol], ps[0:64, 0:nt, 0:ncol], ACTF.Copy)
        nc.gpsimd.tensor_tensor(
            obv, stA[:, 0:nt, 0:ncol], stB[:, 0:nt, 0:ncol], op=OP.add
        )
        return nc.gpsimd.tensor_tensor(obv, obv, gps[:, 0:nt, 0:ncol], op=OP.add)
    return nc.vector.scalar_tensor_tensor(
        obv,
        ps[0:64, 0:nt, 0:ncol],
        b_comb[:],
        stB[:, 0:nt, 0:ncol],
        op0=OP.add,
        op1=OP.add,
    )


def build_program():
    if "nc" in _cache:
        return _cache["nc"]
    nc = bacc.Bacc("TRN2", target_bir_lowering=False, debug=False, enable_asserts=False, enable_partition_id=False)
    xs_ap = nc.dram_tensor("xs", [SPB, 128, FLAT], BF16, kind="ExternalInput").ap()
    wpsA_d = nc.dram_tensor("wpsA", [128, E // 2, 3, 128], BF16, kind="ExternalInput").ap()
    wpsB_d = nc.dram_tensor("wpsB", [128, E // 2, 3, 128], BF16, kind="ExternalInput").ap()
    ident_d = nc.dram_tensor("ident", [128, 128], BF16, kind="ExternalInput").ap()
    ident_d = nc.dram_tensor("ident", [128, 128], BF16, kind="ExternalInput").ap()
    gconst_d = nc.dram_tensor("gconst", [128, 90], F32, kind="ExternalInput").ap()
    out_ap = nc.dram_tensor("out", [SPB, C, H * WP], BF16, kind="ExternalOutput").ap()

    with tile.TileContext(nc) as tc, ExitStack() as ctx:
        pools = {
            "const": ctx.enter_context(tc.tile_pool(name="const", bufs=1)),
            "xx": ctx.enter_context(tc.tile_pool(name="xx", bufs=SPB)),
            "gate": ctx.enter_context(tc.tile_pool(name="gate", bufs=2)),
            "wcomb": ctx.enter_context(tc.tile_pool(name="wcomb", bufs=2)),
            "stage": ctx.enter_context(tc.tile_pool(name="stage", bufs=6)),
            "cpsum": ctx.enter_context(tc.tile_pool(name="cpsum", bufs=3, space="PSUM")),
            "gpsum": ctx.enter_context(tc.tile_pool(name="gpsum", bufs=2, space="PSUM")),
        }
        cp = pools["const"]
        # +4 zeroed pad cols so the tail tile's widened matmul read stays
        # in bounds
        XX0 = pools["xx"].tile([128, FLAT + 4], BF16, tag="XX", name="XX0")
        XX1 = pools["xx"].tile([128, FLAT + 4], BF16, tag="XX", name="XX1")
        nc.vector.memset(XX0[:, FLAT : FLAT + 4], 0.0)
        nc.vector.memset(XX1[:, FLAT : FLAT + 4], 0.0)
        gconst_sb = cp.tile([128, 90], F32)
        ones = cp.tile([1, 128], F32)
        nc.gpsimd.memset(ones[:], 1.0)
        wpsA_sb = cp.tile([128, E // 2, 3, 128], BF16)
        wpsB_sb = cp.tile([128, E // 2, 3, 128], BF16)
        ident_sb = cp.tile([128, 128], BF16)
        ident_sb = cp.tile([128, 128], BF16)
        pools["scrD"] = cp.tile([128, QC + 2], BF16, name="scrD")
        pools["scrS"] = cp.tile([128, QC + 2], BF16, name="scrS")

        # ---- loads + prologue compute, interleaved so each consumer's
        # queue drain covers only the transfers it actually needs (a
        # consumer emitted after later triggers on a lane waits for ALL
        # of them - this drain effect, not bandwidth, dominated the v4/v6
        # prologues) ----
        C3A = 3 * QC + 2113
        nc.scalar.dma_start(wpsA_sb[:], wpsA_d[:])       # warmup needs it
        nc.scalar.dma_start(ident_sb[:], ident_d[:])

        # PE warm-up: no-DMA scratch matmuls start ~2us (HAM to 8/8),
        # then chunk-gated batches self-time the warm window to the load
        warm_sc = cp.tile([128, 384], BF16, name="warm_sc")
        nc.gpsimd.memset(warm_sc[:], 0.25)
        warm_ps = pools["gpsum"].tile([128, 384], F32, tag="cps", name="warm_ps")

        def emit_warm(n, rhs):
            for _ in range(n):
                nc.tensor.matmul(
                    warm_ps[:], lhsT=warm_sc[:, 0:128], rhs=rhs,
                    start=True, stop=True,
                )

        emit_warm(30, warm_sc[:])

        part0 = pools["gate"].tile([128, 2], F32, tag="part", name="part0")
        h_ext0 = pools["gate"].tile([GH + 1, 1], F32, tag="h_ext", name="h_ext0")
        h_ext1 = pools["gate"].tile([GH + 1, 1], F32, tag="h_ext", name="h_ext1")

        nc.sync.dma_start(XX0[:, 0:QC], xs_ap[0, :, 0:QC])
        nc.sync.dma_start(h_ext0[GH : GH + 1, 0:1], ones[0:1, 0:1])
        nc.sync.dma_start(h_ext1[GH : GH + 1, 0:1], ones[0:1, 0:1])
        w1i = _emit_gap_op(nc, pools, XX0, part0, GAP_TOP[0], is_bot=False, eng="act")
        emit_warm(8, XX0[:, 0:384])

        nc.gpsimd.dma_start(gconst_sb[:], gconst_d[:])
        nc.gpsimd.dma_start(XX0[:, QC : 2 * QC], xs_ap[0, :, QC : 2 * QC])
        nc.scalar.dma_start(XX0[:, 2 * QC : 3 * QC], xs_ap[0, :, 2 * QC : 3 * QC])
        w2i = _emit_gap_op(nc, pools, XX0, part0, GAP_TOP[1], is_bot=False, eng="dve")
        w3i = _emit_gap_op(nc, pools, XX0, part0, GAP_BOT[0], is_bot=True, eng="dve")
        emit_warm(8, XX0[:, QC : QC + 384])
        emit_warm(8, XX0[:, 2 * QC : 2 * QC + 384])

        nc.gpsimd.dma_start(XX0[:, C3A:FLAT], xs_ap[0, :, C3A:FLAT])
        nc.sync.dma_start(XX0[:, 3 * QC : C3A], xs_ap[0, :, 3 * QC : C3A])
        w4i = _emit_gap_op(nc, pools, XX0, part0, GAP_BOT[1], is_bot=True, eng="act")
        emit_warm(8, XX0[:, 3 * QC : 3 * QC + 384])

        nc.scalar.dma_start(wpsB_sb[:], wpsB_d[:])

        wg1x2_sb = gconst_sb[:, 0:16]
        bg1_sb = gconst_sb[0:16, 16:17]
        wg2_sb = gconst_sb[0:17, 17:25]
        bexp_sb = gconst_sb[0:8, 25:89]
        consts = (wg1x2_sb, bg1_sb, wg2_sb, bexp_sb, ones)

        pooled0 = pools["gate"].tile([128, 1], F32, tag="pooled", name="pooled0")
        nc.vector.tensor_reduce(pooled0, part0[:], axis=AX.X, op=OP.add)
        wb0, bcomb0 = _emit_gate(nc, pools, 0, pooled0, consts, h_ext0)
        wcombr0 = _emit_mac_pe(nc, pools, 0, wb0, wpsA_sb, wpsB_sb, ident_sb)
        zb = cp.tile([64, 2, 390], BF16, name="zb")
        nc.gpsimd.memset(zb[:], 0.0)
        bB0 = pools["gate"].tile([64, 2, 390], BF16, tag="bB", name="bB0")
        nc.vector.scalar_tensor_tensor(
            bB0[:], zb[:], bcomb0[:], zb[:], op0=OP.add, op1=OP.add
        )

        # ---- s1 x loads: each trigger HARD-pinned after an s0 GAP op so
        # no s0 consumer's queue drain can end up covering s1 transfers
        # (the scheduler otherwise floats these triggers early) ----
        def pin_trig(ti, gi):
            tile.add_dep_helper(
                ti.ins, gi.ins, sync=True,
                reason="s1 load held until s0 GAP consumed its lane",
            )

        pin_trig(nc.gpsimd.dma_start(XX1[:, 0:QC], xs_ap[1, :, 0:QC]), w1i)
        pin_trig(nc.sync.dma_start(XX1[:, QC : 2 * QC], xs_ap[1, :, QC : 2 * QC]), w2i)
        pin_trig(
            nc.scalar.dma_start(XX1[:, 2 * QC : 3 * QC], xs_ap[1, :, 2 * QC : 3 * QC]),
            w3i,
        )
        pin_trig(nc.sync.dma_start(XX1[:, 3 * QC : C3A], xs_ap[1, :, 3 * QC : C3A]), w4i)
        pin_trig(nc.gpsimd.dma_start(XX1[:, C3A:FLAT], xs_ap[1, :, C3A:FLAT]), w4i)

        part1 = pools["gate"].tile([128, 4], F32, tag="part", name="part1")
        nc.gpsimd.memset(part1[0:64, 2:4], 0.0)
        s1_state = {}

        def s1_hook(p, comb):
            def pin(gi):
                tile.add_dep_helper(
                    gi.ins, comb.ins, sync=False,
                    reason="s1 prep slotted after this pair's combine",
                )
            if p == 4:
                pin(_emit_gap_op(nc, pools, XX1, part1, GAP_TOP[0], is_bot=False, eng="act"))
            elif p == 7:
                pin(_emit_gap_op(nc, pools, XX1, part1, GAP_TOP[1], is_bot=False, eng="dve"))
            elif p in (9, 11, 13, 14):
                k = {9: 0, 11: 1, 13: 2, 14: 3}[p]
                pin(_emit_gap_op(nc, pools, XX1, part1, GAP_BOT4[k], is_bot=True, eng="dve"))
            elif p == 15:
                pooled1 = pools["gate"].tile(
                    [128, 1], F32, tag="pooled", name="pooled1"
                )
                pin(nc.vector.tensor_reduce(pooled1, part1[:], axis=AX.X, op=OP.add))
                wb1, bcomb1 = _emit_gate(nc, pools, 1, pooled1, consts, h_ext1)
                s1_state["bcomb"] = bcomb1
                s1_state["wcombr"] = _emit_mac_pe(
                    nc, pools, 1, wb1, wpsA_sb, wpsB_sb, ident_sb
                )
                bB1 = pools["gate"].tile([64, 2, 390], BF16, tag="bB", name="bB1")
                nc.vector.scalar_tensor_tensor(
                    bB1[:], zb[:], bcomb1[:], zb[:], op0=OP.add, op1=OP.add
                )
                s1_state["bB"] = bB1

        # out batching: one [64, OBW] buffer per 24-row batch (batch 5 is
        # 8 rows); s0 batches drain on SP, s1 batches on gpsimd
        obstate = {0: [None, 0], 1: [None, 0]}

        bBmap = {}

        def emit_sample_pairs(s, XX, wcombr, bcomb, rng, hook=None):
            for p in rng:
                batch = min(p // 4, 5)
                ob, ocol = obstate[s]
                if ob is None:
                    ob = pools["stage"].tile(
                        [64, OBW], BF16, tag="ob", name=f"ob{s}_{batch}", bufs=3
                    )
                    obstate[s] = [ob, 0]
                    ocol = 0
                gps = bBmap.get(s) if (s == 0 and p in GPS_PAIRS_S0) else None
                comb = _emit_pair(nc, pools, s, p, XX, wcombr, bcomb, ob, ocol, gps)
                ocol += 780 if p < NPAIR - 1 else 260
                obstate[s][1] = ocol
                bcols = OBW if batch < 5 else 1040
                if ocol == bcols:
                    lane = nc.sync if s == 0 else nc.gpsimd
                    lane.dma_start(
                        out_ap[s, :, 24 * batch * WP : 24 * batch * WP + bcols],
                        ob[:, 0:bcols],
                    )
                    obstate[s] = [None, 0]
                if hook is not None:
                    hook(p, comb)

        bBmap[0] = bB0
        emit_sample_pairs(0, XX0, wcombr0, bcomb0, range(NPAIR), s1_hook)
        emit_sample_pairs(
            1, XX1, s1_state["wcombr"], s1_state["bcomb"], range(NPAIR)
        )

    nc.compile()
    _cache["nc"] = nc
    return nc


def host_prep(x, wg1, bg1, wg2, bg2, w_exp, b_exp):
    """Host-side layout prep + per-core sharding. Returns in_maps list."""
    x = np.asarray(x, dtype=np.float32)
    wg1 = np.asarray(wg1, dtype=np.float32)
    bg1 = np.asarray(bg1, dtype=np.float32)
    wg2 = np.asarray(wg2, dtype=np.float32)
    bg2 = np.asarray(bg2, dtype=np.float32)
    w_exp = np.asarray(w_exp, dtype=np.float32)
    b_exp = np.asarray(b_exp, dtype=np.float32)

    # x shipped as [B, 128, FLAT] bf16: rows 0:64 = zero-padded flat
    # image, rows 64:128 = the same shifted +2 elements (the conv's
    # bottom-half K copy) — both SBUF halves land in one full-rate DMA
    xpad = np.zeros((B, C, HP, WP), np.float32)
    xpad[:, :, 1 : H + 1, 1 : W + 1] = x
    flat = xpad.reshape(B, C, FLAT)
    xs = np.zeros((B, 128, FLAT), NPBF16)
    xs[:, 0:64] = flat.astype(NPBF16)
    xs[:, 64:128, 0 : FLAT - 2] = flat[:, :, 2:].astype(NPBF16)

    # wps [128, E, 3(dy), 128]: K top/bottom = taps dx 0/2 on M 0:64 (A),
    # center dx=1 on M 64:128 top (B, bottom zero). Residual identity is
    # folded into every expert's center tap (sum of probs is ~1).
    wt = np.transpose(w_exp, (2, 0, 3, 4, 1))  # [I, E, dy, dx, O]
    wps = np.zeros((128, E, 3, 128), np.float32)
    wps[0:64, :, :, 0:64] = wt[:, :, :, 0, :]
    wps[64:128, :, :, 0:64] = wt[:, :, :, 2, :]
    wps[0:64, :, :, 64:128] = wt[:, :, :, 1, :]
    ii = np.arange(64)
    wps[ii, :, 1, 64 + ii] += 1.0

    gconst = np.zeros((128, 90), np.float32)
    gconst[:, 0:16] = np.concatenate([wg1, wg1], axis=0) / (H * W)
    gconst[0:16, 16] = bg1
    gconst[0:16, 17:25] = wg2
    gconst[16, 17:25] = bg2
    gconst[0:8, 25:89] = b_exp

    shared = {
        "wpsA": np.ascontiguousarray(wps[:, 0:4]).astype(NPBF16),
        "wpsB": np.ascontiguousarray(wps[:, 4:8]).astype(NPBF16),
        "ident": np.eye(128, dtype=NPBF16),
        "gconst": gconst,
    }
    return [
        {"xs": np.ascontiguousarray(xs[SPB * k : SPB * (k + 1)]), **shared}
        for k in range(NCORES)
    ]


def _decode_out(o):
    """[C, H*WP] bf16 -> [C, H, W] f32 (strip the pad columns)."""
    return np.asarray(o, dtype=np.float32).reshape(C, H, WP)[:, :, 0:W]


def kernel(x, wg1, bg1, wg2, bg2, w_exp, b_exp):
    nc = build_program()
    in_maps = host_prep(x, wg1, bg1, wg2, bg2, w_exp, b_exp)
    res = run_bass_kernel_spmd(nc, in_maps, list(range(NCORES)))
    out = np.empty((B, C, H, W), np.float32)
    for k in range(NCORES):
        o = np.asarray(res.results[k]["out"])
        for s in range(SPB):
            out[SPB * k + s] = _decode_out(o[s])
    return out
